# revision 1
# baseline (speedup 1.0000x reference)
import sys
import numpy as np

sys.path.insert(0, "/opt/trn_rl_repo")
sys.path.insert(0, "/opt/trn_rl_repo/concourse")

import concourse.bass as bass
import concourse.bacc as bacc
import concourse.mybir as mybir
import concourse.tile as tile
from concourse.bass import IndirectOffsetOnAxis
from concourse.bass_utils import run_bass_kernel_spmd
from concourse.masks import make_identity

F32 = mybir.dt.float32
I32 = mybir.dt.int32
I16 = mybir.dt.int16

N = 20000
E = 160000
B = 128
NDEV = 8
NPD = N // NDEV          # 2500 nodes per device
NT = (NPD + 127) // 128  # 20 dst tiles per device
H = 4
C1IN, C1 = 768, 512
C2IN, C2 = 512, 256
XROW = 832               # [x(768) | as1(4) | ad1(4) | pad] -> 256B-aligned row
T2W = 576                # [h1(512) | as2(4) | ad2(4) | pad] -> 256B-aligned row
NEG = 0.2
USE_DMA_GATHER = False


def _host_prep(edge_index, batch):
    """Integer-only preprocessing: edge partitioning, sorting, chunk layout."""
    src = np.concatenate([edge_index[0], np.arange(N, dtype=np.int64)]).astype(np.int64)
    dst = np.concatenate([edge_index[1], np.arange(N, dtype=np.int64)]).astype(np.int64)
    order = np.argsort(dst, kind="stable")
    src, dst = src[order], dst[order]

    dev = dst // NPD
    tloc = (dst % NPD) // 128
    cnt = np.zeros((NDEV, NT), dtype=np.int64)
    for d in range(NDEV):
        m = dev == d
        cnt[d] = np.bincount(tloc[m], minlength=NT)
    Ks = [max(1, int(np.ceil(cnt[:, t].max() / 128.0))) for t in range(NT)]
    SK = sum(Ks)
    offs = np.cumsum([0] + Ks)

    # flat src index per (device, tile): edge j -> lane j%128, chunk j//128
    xidx16 = np.zeros((NDEV, 128, 8 * SK), dtype=np.int16)  # wrapped for dma_gather
    xidx32 = np.zeros((NDEV, 128, SK), dtype=np.int32)      # per-chunk indirect layout
    dstf = np.full((NDEV, 128, SK), -1.0, dtype=np.float32)
    dstfR = np.full((NDEV, SK, 128), -1.0, dtype=np.float32)  # chunk-major rows

    for d in range(NDEV):
        m = dev == d
        s_d, t_d, dl_d = src[m], tloc[m], (dst[m] % NPD) % 128
        for t in range(NT):
            mt = t_d == t
            s_t = s_d[mt]
            dl_t = dl_d[mt]
            K = Ks[t]
            o = offs[t]
            flat = np.zeros(K * 128, dtype=np.int64)
            flat[:len(s_t)] = s_t
            # wrapped int16 for dma_gather: index j at [j%16, j//16], tiled x8
            w = np.tile(flat.reshape(8 * K, 16).T, (8, 1)).astype(np.int16)
            xidx16[d, :, 8 * o: 8 * (o + K)] = w
            j = np.arange(len(s_t))
            xidx32[d, j % 128, o + j // 128] = s_t
            dstf[d, j % 128, o + j // 128] = dl_t.astype(np.float32)
            dstfR[d, o + j // 128, j % 128] = dl_t.astype(np.float32)

    batchf = np.full((NDEV, 128, NT), -1.0, dtype=np.float32)
    b_np = np.asarray(batch).astype(np.int64)
    for d in range(NDEV):
        for t in range(NT):
            rows = min(128, NPD - t * 128)
            g = b_np[d * NPD + t * 128: d * NPD + t * 128 + rows]
            batchf[d, :rows, t] = g.astype(np.float32)

    return Ks, offs, SK, xidx16, xidx32, dstf, dstfR, batchf


def _build_A(a_src, a_dst, cph):
    A = np.zeros((H * cph, 8), dtype=np.float32)
    for h in range(H):
        A[h * cph:(h + 1) * cph, h] = a_src[h]
        A[h * cph:(h + 1) * cph, 4 + h] = a_dst[h]
    return A


def _build(Ks, offs, SK):
    """Emit the Bass program (identical for all 8 cores)."""
    nc = bacc.Bacc("TRN2", target_bir_lowering=False, debug=False, num_devices=NDEV)

    # ---- I/O ----
    xa_t = nc.dram_tensor("xa", [N, XROW], F32, kind="ExternalInput")
    xlocT_t = nc.dram_tensor("xlocT", [C1IN, NPD], F32, kind="ExternalInput")
    W1_t = nc.dram_tensor("W1", [C1IN, H * C1], F32, kind="ExternalInput")
    W1T_t = nc.dram_tensor("W1T", [H * C1, C1IN], F32, kind="ExternalInput")
    W2_t = nc.dram_tensor("W2", [C2IN, H * C2], F32, kind="ExternalInput")
    W2T_t = nc.dram_tensor("W2T", [H * C2, C2IN], F32, kind="ExternalInput")
    A1_t = nc.dram_tensor("A1", [H * C1, 8], F32, kind="ExternalInput")
    A2_t = nc.dram_tensor("A2", [H * C2, 8], F32, kind="ExternalInput")
    b1_t = nc.dram_tensor("b1", [C1], F32, kind="ExternalInput")
    b2_t = nc.dram_tensor("b2", [C2], F32, kind="ExternalInput")
    fcW_t = nc.dram_tensor("fcW", [C2, 2], F32, kind="ExternalInput")
    fcb_t = nc.dram_tensor("fcb", [2], F32, kind="ExternalInput")
    xidx16_t = nc.dram_tensor("xidx16", [128, 8 * SK], I16, kind="ExternalInput")
    xidx32_t = nc.dram_tensor("xidx32", [128, SK], I32, kind="ExternalInput")
    dstf_t = nc.dram_tensor("dstf", [128, SK], F32, kind="ExternalInput")
    dstfR_t = nc.dram_tensor("dstfR", [SK, 128], F32, kind="ExternalInput")
    batchf_t = nc.dram_tensor("batchf", [128, NT], F32, kind="ExternalInput")
    y_t = nc.dram_tensor("y", [B, 2], F32, kind="ExternalOutput")

    # ---- internal DRAM ----
    asad1_loc = nc.dram_tensor("asad1_loc", [NPD, 8], F32)
    asad1_full = nc.dram_tensor("asad1_full", [N, 8], F32, addr_space="Shared")
    t2_loc = nc.dram_tensor("t2_loc", [NPD, T2W], F32)
    t2_full = nc.dram_tensor("t2_full", [N, T2W], F32, addr_space="Shared")
    pc_loc = nc.dram_tensor("pc_loc", [B, C2 + 1], F32)
    pc_red = nc.dram_tensor("pc_red", [B, C2 + 1], F32, addr_space="Shared")

    RG = [list(range(NDEV))]
    KMAX = max(Ks)

    import os as _os
    DEBUG = bool(int(_os.environ.get("BASS_GAT_DEBUG", "0")))
    if DEBUG:
        dbg_asad1 = nc.dram_tensor("dbg_asad1", [N, 8], F32, kind="ExternalOutput")
        dbg_t2 = nc.dram_tensor("dbg_t2", [N, T2W], F32, kind="ExternalOutput")
        dbg_pc = nc.dram_tensor("dbg_pc", [B, C2 + 1], F32, kind="ExternalOutput")
        dbg_den = nc.dram_tensor("dbg_den", [NT, 128, 4], F32, kind="ExternalOutput")
        dbg_alpha = nc.dram_tensor("dbg_alpha", [NT, 128, 4 * KMAX], F32, kind="ExternalOutput")
        dbg_xg = nc.dram_tensor("dbg_xg", [NT, 128, XROW], F32, kind="ExternalOutput")

    with tile.TileContext(nc) as tc:
        with (
            tc.tile_pool(name="const", bufs=1) as cp,
            tc.tile_pool(name="small", bufs=2) as sp,
            tc.tile_pool(name="selp", bufs=KMAX + 1) as selp,
            tc.tile_pool(name="selTp", bufs=KMAX + 1) as selTp,
            tc.tile_pool(name="selwp", bufs=3) as selwp,
            tc.tile_pool(name="xgp", bufs=2) as xgp,
            tc.tile_pool(name="utp", bufs=2) as utp,
            tc.tile_pool(name="psu", bufs=1, space="PSUM") as psu,
            tc.tile_pool(name="psh", bufs=1, space="PSUM") as psh,
            tc.tile_pool(name="psr", bufs=1, space="PSUM") as psr,
        ):
            # ================= constants =================
            ident = cp.tile([128, 128], F32, tag="ident")
            make_identity(nc, ident[:])
            iota_i = cp.tile([128, 128], I32, tag="iota_i")
            nc.gpsimd.iota(iota_i[:], pattern=[[1, 128]], base=0, channel_multiplier=0)
            iotaT = cp.tile([128, 128], F32, tag="iotaT")
            nc.vector.tensor_copy(out=iotaT[:], in_=iota_i[:])
            iota_ci = cp.tile([128, 1], I32, tag="iota_ci")
            nc.gpsimd.iota(iota_ci[:], pattern=[[1, 1]], base=0, channel_multiplier=1)
            iotaC = cp.tile([128, 1], F32, tag="iotaC")
            nc.vector.tensor_copy(out=iotaC[:], in_=iota_ci[:])
            ones1 = cp.tile([1, 128], F32, tag="ones1")
            nc.vector.memset(ones1[:], 1.0)

            b1_sb = cp.tile([1, C1], F32, tag="b1")
            nc.sync.dma_start(out=b1_sb[:], in_=b1_t[None, :])
            b2_sb = cp.tile([1, C2], F32, tag="b2")
            nc.sync.dma_start(out=b2_sb[:], in_=b2_t[None, :])
            fcb_sb = cp.tile([1, 2], F32, tag="fcb")
            nc.sync.dma_start(out=fcb_sb[:], in_=fcb_t[None, :])
            fcW_sb = cp.tile([128, 4], F32, tag="fcW")
            for c in range(2):
                nc.sync.dma_start(out=fcW_sb[:, 2 * c:2 * c + 2],
                                  in_=fcW_t[c * 128:(c + 1) * 128, :])

            CC1 = C1IN // 128  # 6
            CC2 = C2IN // 128  # 4
            W1_sb = cp.tile([128, CC1 * H * C1], F32, tag="W1")
            for c in range(CC1):
                nc.sync.dma_start(out=W1_sb[:, c * H * C1:(c + 1) * H * C1],
                                  in_=W1_t[c * 128:(c + 1) * 128, :])
            W2_sb = cp.tile([128, CC2 * H * C2], F32, tag="W2")
            for c in range(CC2):
                nc.sync.dma_start(out=W2_sb[:, c * H * C2:(c + 1) * H * C2],
                                  in_=W2_t[c * 128:(c + 1) * 128, :])

            # ================= wa1 / wa2 =================
            wa1_sb = cp.tile([128, CC1 * 8], F32, tag="wa1")
            wa2_sb = cp.tile([128, CC2 * 8], F32, tag="wa2")
            with tc.tile_pool(name="prep", bufs=2) as pp:
                def compute_wa(WT_t, A_t, M, CC, CIN, wa_sb):
                    A_sb = pp.tile([128, M * 8], F32, tag="a")
                    nc.sync.dma_start(
                        out=A_sb[:, :M * 8].rearrange("p (m j) -> p m j", j=8),
                        in_=A_t[:].rearrange("(m p) j -> p m j", p=128))
                    nc.vector.memset(wa_sb[:], 0.0)
                    for m in range(M):
                        WTm = pp.tile([128, C1IN], F32, tag="wtm")
                        nc.sync.dma_start(out=WTm[:, :CIN],
                                          in_=WT_t[m * 128:(m + 1) * 128, :])
                        for c in range(CC):
                            ps = psr.tile([128, 128], F32, tag="rot")
                            nc.tensor.matmul(out=ps[:, :8],
                                             lhsT=WTm[:, c * 128:(c + 1) * 128],
                                             rhs=A_sb[:, m * 8:(m + 1) * 8],
                                             start=True, stop=True)
                            nc.vector.tensor_tensor(out=wa_sb[:, c * 8:(c + 1) * 8],
                                                    in0=wa_sb[:, c * 8:(c + 1) * 8],
                                                    in1=ps[:, :8],
                                                    op=mybir.AluOpType.add)

                compute_wa(W1T_t, A1_t, H * C1 // 128, CC1, C1IN, wa1_sb)
                compute_wa(W2T_t, A2_t, H * C2 // 128, CC2, C2IN, wa2_sb)

                # ============ asad1 = x_loc @ wa1 ============
                for t in range(NT):
                    rows = min(128, NPD - t * 128)
                    xT = pp.tile([128, CC1 * rows], F32, tag="xT")
                    nc.sync.dma_start(
                        out=xT[:].rearrange("p (c n) -> p c n", c=CC1),
                        in_=xlocT_t[:, t * 128: t * 128 + rows].rearrange(
                            "(c p) n -> p c n", p=128))
                    ps = psr.tile([128, 128], F32, tag="rot")
                    for c in range(CC1):
                        nc.tensor.matmul(out=ps[:rows, :8],
                                         lhsT=xT[:, c * rows:(c + 1) * rows],
                                         rhs=wa1_sb[:, c * 8:(c + 1) * 8],
                                         start=(c == 0), stop=(c == CC1 - 1))
                    as1 = sp.tile([128, 8], F32, tag="as1")
                    nc.vector.tensor_copy(out=as1[:rows, :], in_=ps[:rows, :8])
                    nc.sync.dma_start(out=asad1_loc[t * 128: t * 128 + rows, :],
                                      in_=as1[:rows, :])

            nc.gpsimd.collective_compute(
                "AllGather", mybir.AluOpType.bypass, replica_groups=RG,
                ins=[asad1_loc[:, :]], outs=[asad1_full[:, :]])
            # write as1/ad1 into the gather table columns 768:776
            nc.sync.dma_start(out=xa_t[:, C1IN:C1IN + 8], in_=asad1_full[:, :])

            # ================= helper: one GAT layer sweep =================
            def gat_sweep(layer):
                if layer == 1:
                    CIN, CC, COUT = C1IN, CC1, C1
                    ROW = XROW
                    W_sb, b_sb = W1_sb, b1_sb
                    x_tab = xa_t
                    ad_loc, ad_col = asad1_loc, 4
                else:
                    CIN, CC, COUT = C2IN, CC2, C2
                    ROW = T2W
                    W_sb, b_sb = W2_sb, b2_sb
                    x_tab = t2_full
                    ad_loc, ad_col = t2_loc, C2IN + 4

                poolacc = None
                if layer == 2:
                    poolacc = cp.tile([128, C2 + 1], F32, tag="poolacc")
                    nc.vector.memset(poolacc[:], 0.0)

                for t in range(NT):
                    K = Ks[t]
                    o = offs[t]
                    rows = min(128, NPD - t * 128)

                    # ---------- gather x rows (+ embedded as cols) ----------
                    xg = xgp.tile([128, KMAX * XROW], F32, tag="xg")
                    if USE_DMA_GATHER:
                        idx_sb = sp.tile([128, 8 * KMAX], I16, tag="idx16")
                        nc.sync.dma_start(out=idx_sb[:, :8 * K],
                                          in_=xidx16_t[:, 8 * o:8 * (o + K)])
                        nc.gpsimd.dma_gather(
                            out_ap=xg[:, :K * ROW].rearrange("p (k d) -> p k d", d=ROW),
                            in_ap=x_tab[:, :],
                            idxs_ap=idx_sb[:, :8 * K],
                            num_idxs=K * 128, num_idxs_reg=K * 128,
                            elem_size=ROW)
                    else:
                        idx_sb = sp.tile([128, KMAX], I32, tag="idx32")
                        nc.sync.dma_start(out=idx_sb[:, :K],
                                          in_=xidx32_t[:, o:o + K])
                        for k in range(K):
                            nc.gpsimd.indirect_dma_start(
                                out=xg[:, k * ROW:(k + 1) * ROW], out_offset=None,
                                in_=x_tab[:, :],
                                in_offset=IndirectOffsetOnAxis(
                                    ap=idx_sb[:, k:k + 1], axis=0))

                    dstf_sb = sp.tile([128, KMAX], F32, tag="dstf")
                    nc.sync.dma_start(out=dstf_sb[:, :K], in_=dstf_t[:, o:o + K])
                    ad_sb = sp.tile([128, 4], F32, tag="ad")
                    if rows < 128:
                        nc.vector.memset(ad_sb[:], 0.0)
                    nc.sync.dma_start(out=ad_sb[:rows, :],
                                      in_=ad_loc[t * 128:t * 128 + rows,
                                                 ad_col:ad_col + 4])

                    # ---------- selection matrices ----------
                    # sel[e, d] = (dst_local[e] == d); selT built directly from a
                    # partition-broadcast DMA of the chunk's dst row (no PE transpose)
                    sels = []
                    selTs = []
                    for k in range(K):
                        sel = selp.tile([128, 128], F32, tag="sel")
                        nc.vector.tensor_tensor(
                            out=sel[:],
                            in0=dstf_sb[:, k:k + 1].to_broadcast([128, 128]),
                            in1=iotaT[:], op=mybir.AluOpType.is_equal)
                        dstfT = sp.tile([128, 128], F32, tag="dstfT")
                        nc.sync.dma_start(
                            out=dstfT[:],
                            in_=dstfR_t[o + k:o + k + 1, :].to_broadcast([128, 128]))
                        selT = selTp.tile([128, 128], F32, tag="selT")
                        nc.vector.tensor_tensor(
                            out=selT[:], in0=iotaC[:, :1].to_broadcast([128, 128]),
                            in1=dstfT[:], op=mybir.AluOpType.is_equal)
                        sels.append(sel)
                        selTs.append(selT)

                    # ---------- ad per edge; e = lrelu(as+ad); ex=exp ----------
                    ade_ps = psh.tile([128, 4 * KMAX], F32, tag="hold")
                    for k in range(K):
                        nc.tensor.matmul(out=ade_ps[:, 4 * k:4 * k + 4],
                                         lhsT=selTs[k][:], rhs=ad_sb[:],
                                         start=(k == 0), stop=(k == K - 1))
                    xg3 = xg[:, :K * ROW].rearrange("p (k d) -> p k d", d=ROW)
                    z = sp.tile([128, 4 * KMAX], F32, tag="z")
                    z3 = z[:, :4 * K].rearrange("p (k s) -> p k s", s=4)
                    nc.vector.tensor_tensor(out=z3, in0=xg3[:, :, CIN:CIN + 4],
                                            in1=ade_ps[:, :4 * K].rearrange(
                                                "p (k s) -> p k s", s=4),
                                            op=mybir.AluOpType.add)
                    zs = sp.tile([128, 4 * KMAX], F32, tag="zs")
                    nc.vector.tensor_scalar_mul(zs[:, :4 * K], z[:, :4 * K], NEG)
                    nc.vector.tensor_tensor(out=z[:, :4 * K], in0=z[:, :4 * K],
                                            in1=zs[:, :4 * K], op=mybir.AluOpType.max)
                    ex = sp.tile([128, 4 * KMAX], F32, tag="ex")
                    nc.scalar.activation(out=ex[:, :4 * K], in_=z[:, :4 * K],
                                         func=mybir.ActivationFunctionType.Exp)

                    # ---------- softmax denominator ----------
                    den_ps = psh.tile([128, 4], F32, tag="hold")
                    for k in range(K):
                        nc.tensor.matmul(out=den_ps[:], lhsT=sels[k][:],
                                         rhs=ex[:, 4 * k:4 * k + 4],
                                         start=(k == 0), stop=(k == K - 1))
                    den_sb = sp.tile([128, 4], F32, tag="den")
                    nc.vector.tensor_copy(out=den_sb[:], in_=den_ps[:])
                    dene_ps = psh.tile([128, 4 * KMAX], F32, tag="hold")
                    for k in range(K):
                        nc.tensor.matmul(out=dene_ps[:, 4 * k:4 * k + 4],
                                         lhsT=selTs[k][:], rhs=den_sb[:],
                                         start=(k == 0), stop=(k == K - 1))
                    rden = sp.tile([128, 4 * KMAX], F32, tag="rden")
                    nc.vector.tensor_scalar(out=rden[:, :4 * K], in0=dene_ps[:, :4 * K],
                                            scalar1=4.0, scalar2=1e-20,
                                            op0=mybir.AluOpType.mult,
                                            op1=mybir.AluOpType.max)
                    nc.vector.reciprocal(out=rden[:, :4 * K], in_=rden[:, :4 * K])
                    alpha = sp.tile([128, 4 * KMAX], F32, tag="alpha")
                    nc.vector.tensor_tensor(out=alpha[:, :4 * K], in0=ex[:, :4 * K],
                                            in1=rden[:, :4 * K],
                                            op=mybir.AluOpType.mult)
                    if DEBUG and layer == 1:
                        nc.sync.dma_start(out=dbg_den[t, :, :], in_=den_sb[:])
                        nc.sync.dma_start(out=dbg_alpha[t, :, :4 * K],
                                          in_=alpha[:, :4 * K])
                        nc.sync.dma_start(out=dbg_xg[t, :, :], in_=xg[:, :XROW])

                    # ---------- aggregate (transposed) ----------
                    ut_ps = psu.tile([128, CC1 * 512], F32, tag="ut")
                    for k in range(K):
                        selw = selwp.tile([128, 512], F32, tag="selw")
                        nc.vector.tensor_tensor(
                            out=selw[:].rearrange("p (h d) -> p h d", d=128),
                            in0=sels[k][:, None, :].to_broadcast([128, 4, 128]),
                            in1=alpha[:, 4 * k:4 * k + 4][:, :, None].to_broadcast(
                                [128, 4, 128]),
                            op=mybir.AluOpType.mult)
                        for c in range(CC):
                            nc.tensor.matmul(
                                out=ut_ps[:, c * 512:(c + 1) * 512],
                                lhsT=xg[:, k * ROW + c * 128: k * ROW + (c + 1) * 128],
                                rhs=selw[:],
                                start=(k == 0), stop=(k == K - 1))

                    ut_sb = utp.tile([128, CC1 * 512], F32, tag="ut")
                    nc.vector.tensor_copy(out=ut_sb[:, :CC * 512],
                                          in_=ut_ps[:, :CC * 512])

                    # ---------- output transform + bias ----------
                    out_ps = psh.tile([128, COUT], F32, tag="hold")
                    first = True
                    for h in range(H):
                        for c in range(CC):
                            nc.tensor.matmul(
                                out=out_ps[:],
                                lhsT=ut_sb[:, c * 512 + h * 128: c * 512 + (h + 1) * 128],
                                rhs=W_sb[:, c * H * COUT + h * COUT:
                                         c * H * COUT + (h + 1) * COUT],
                                start=first, stop=False)
                            first = False
                    nc.tensor.matmul(out=out_ps[:], lhsT=ones1[:], rhs=b_sb[:],
                                     start=False, stop=True)

                    if layer == 1:
                        h1_sb = sp.tile([128, C1], F32, tag="h1")
                        nc.vector.tensor_copy(out=h1_sb[:], in_=out_ps[:])
                        # asad2 = h1 @ wa2 via PE transposes of h1
                        as2_ps = psh.tile([128, 8], F32, tag="hold")
                        for c in range(CC2):
                            tp = psr.tile([128, 128], F32, tag="rot")
                            nc.tensor.transpose(out=tp[:],
                                                in_=h1_sb[:, c * 128:(c + 1) * 128],
                                                identity=ident[:])
                            h1T = sp.tile([128, 128], F32, tag="h1T")
                            nc.vector.tensor_copy(out=h1T[:], in_=tp[:])
                            nc.tensor.matmul(out=as2_ps[:], lhsT=h1T[:],
                                             rhs=wa2_sb[:, c * 8:(c + 1) * 8],
                                             start=(c == 0), stop=(c == CC2 - 1))
                        as2_sb = sp.tile([128, 8], F32, tag="as2")
                        nc.vector.tensor_copy(out=as2_sb[:], in_=as2_ps[:])
                        nc.sync.dma_start(out=t2_loc[t * 128: t * 128 + rows, 0:C2IN],
                                          in_=h1_sb[:rows, :])
                        nc.sync.dma_start(
                            out=t2_loc[t * 128: t * 128 + rows, C2IN:C2IN + 8],
                            in_=as2_sb[:rows, :])
                    else:
                        h2_sb = sp.tile([128, C2 + 1], F32, tag="h2")
                        nc.vector.tensor_copy(out=h2_sb[:, :C2], in_=out_ps[:])
                        nc.vector.memset(h2_sb[:, C2:C2 + 1], 1.0)
                        selB = sp.tile([128, 128], F32, tag="selB")
                        nc.vector.tensor_tensor(
                            out=selB[:],
                            in0=batchf_sb[:, t:t + 1].to_broadcast([128, 128]),
                            in1=iotaT[:], op=mybir.AluOpType.is_equal)
                        pc_ps = psr.tile([128, C2 + 1], F32, tag="rot")
                        nc.tensor.matmul(out=pc_ps[:], lhsT=selB[:], rhs=h2_sb[:],
                                         start=True, stop=True)
                        nc.vector.tensor_tensor(out=poolacc[:], in0=poolacc[:],
                                                in1=pc_ps[:], op=mybir.AluOpType.add)
                return poolacc

            # ================= layer 1 =================
            gat_sweep(1)
            nc.gpsimd.collective_compute(
                "AllGather", mybir.AluOpType.bypass, replica_groups=RG,
                ins=[t2_loc[:, :]], outs=[t2_full[:, :]])

            if DEBUG:
                nc.sync.dma_start(out=dbg_asad1[:, :], in_=asad1_full[:, :])
                nc.sync.dma_start(out=dbg_t2[:, :], in_=t2_full[:, :])

            # ================= layer 2 + pooling =================
            batchf_sb = cp.tile([128, NT], F32, tag="batchf")
            nc.sync.dma_start(out=batchf_sb[:], in_=batchf_t[:, :])
            poolacc = gat_sweep(2)

            # ================= pool reduce + FC =================
            nc.sync.dma_start(out=pc_loc[:, :], in_=poolacc[:])
            nc.gpsimd.collective_compute(
                "AllReduce", mybir.AluOpType.add, replica_groups=RG,
                ins=[pc_loc[:, :]], outs=[pc_red[:, :]])
            pc_sb = sp.tile([128, C2 + 1], F32, tag="pc")
            nc.sync.dma_start(out=pc_sb[:], in_=pc_red[:, :])
            cnt = sp.tile([128, 1], F32, tag="cnt")
            nc.vector.tensor_scalar_max(cnt[:], pc_sb[:, C2:C2 + 1], 1.0)
            nc.vector.reciprocal(out=cnt[:], in_=cnt[:])
            g_sb = sp.tile([128, C2], F32, tag="g")
            nc.vector.tensor_scalar_mul(g_sb[:], pc_sb[:, :C2], cnt[:, :1])

            y_ps = psh.tile([128, 2], F32, tag="hold")
            for c in range(2):
                tp = psr.tile([128, 128], F32, tag="rot")
                nc.tensor.transpose(out=tp[:], in_=g_sb[:, c * 128:(c + 1) * 128],
                                    identity=ident[:])
                gT = sp.tile([128, 128], F32, tag="gT")
                nc.vector.tensor_copy(out=gT[:], in_=tp[:])
                nc.tensor.matmul(out=y_ps[:], lhsT=gT[:],
                                 rhs=fcW_sb[:, 2 * c:2 * c + 2],
                                 start=(c == 0), stop=False)
            nc.tensor.matmul(out=y_ps[:], lhsT=ones1[:], rhs=fcb_sb[:],
                             start=False, stop=True)
            y_sb = sp.tile([128, 2], F32, tag="y")
            nc.vector.tensor_copy(out=y_sb[:], in_=y_ps[:])
            nc.sync.dma_start(out=y_t[:, :], in_=y_sb[:])
            if DEBUG:
                nc.sync.dma_start(out=dbg_pc[:, :], in_=pc_red[:, :])

    nc.compile()
    return nc


def _setup_ntff_hook():
    """The image's antenv lacks axon_hooks; synthesize it and register the
    ctypes NTFF profiling hook so trace=True works."""
    import types
    import antenv
    if hasattr(antenv, "axon_hooks"):
        return
    mod = types.ModuleType("antenv.axon_hooks")
    state = {"hook": None}
    mod.set_axon_ntff_profile_hook = lambda h: state.__setitem__("hook", h)
    mod.get_axon_ntff_profile_hook = lambda: state["hook"]
    sys.modules["antenv.axon_hooks"] = mod
    antenv.axon_hooks = mod
    try:
        from trn_agent_boot.trn_boot import _ntff_profile_via_ctypes
        hook = _ntff_profile_via_ctypes("/opt/axon/libaxon_pjrt.so")
        mod.set_axon_ntff_profile_hook(hook)
    except Exception as e:
        print("ntff hook setup failed:", e)


_CACHE = {}


def kernel(**inputs):
    x = np.ascontiguousarray(np.asarray(inputs["x"], dtype=np.float32))
    edge_index = np.asarray(inputs["edge_index"])
    batch = np.asarray(inputs["batch"])
    W1 = np.ascontiguousarray(np.asarray(inputs["W1"], dtype=np.float32))
    W2 = np.ascontiguousarray(np.asarray(inputs["W2"], dtype=np.float32))
    a_src1 = np.asarray(inputs["a_src1"], dtype=np.float32)
    a_dst1 = np.asarray(inputs["a_dst1"], dtype=np.float32)
    a_src2 = np.asarray(inputs["a_src2"], dtype=np.float32)
    a_dst2 = np.asarray(inputs["a_dst2"], dtype=np.float32)
    b1 = np.asarray(inputs["b1"], dtype=np.float32)
    b2 = np.asarray(inputs["b2"], dtype=np.float32)
    fcW = np.ascontiguousarray(np.asarray(inputs["fcW"], dtype=np.float32))
    fcb = np.asarray(inputs["fcb"], dtype=np.float32)

    Ks, offs, SK, xidx16, xidx32, dstf, dstfR, batchf = _host_prep(edge_index, batch)

    key = (tuple(Ks),)
    if key not in _CACHE:
        _CACHE[key] = _build(Ks, offs, SK)
    nc = _CACHE[key]

    W1T = np.ascontiguousarray(W1.T)
    W2T = np.ascontiguousarray(W2.T)
    A1 = _build_A(a_src1, a_dst1, C1)
    A2 = _build_A(a_src2, a_dst2, C2)
    xa = np.zeros((N, XROW), dtype=np.float32)
    xa[:, :C1IN] = x

    in_maps = []
    for d in range(NDEV):
        xloc = x[d * NPD:(d + 1) * NPD]
        in_maps.append({
            "xa": xa, "xlocT": np.ascontiguousarray(xloc.T),
            "W1": W1, "W1T": W1T, "W2": W2, "W2T": W2T,
            "A1": A1, "A2": A2, "b1": b1, "b2": b2,
            "fcW": fcW, "fcb": fcb,
            "xidx16": xidx16[d], "xidx32": xidx32[d], "dstf": dstf[d],
            "dstfR": dstfR[d], "batchf": batchf[d],
        })

    import os as _os
    trace = bool(int(_os.environ.get("BASS_GAT_TRACE", "0")))
    kwargs = {}
    if trace:
        _setup_ntff_hook()
        kwargs = dict(trace=True, trace_cores=[0])
    res = run_bass_kernel_spmd(nc, in_maps, core_ids=list(range(NDEV)), **kwargs)
    if trace:
        kernel.last_exec_ns = res.exec_time_ns
        kernel.last_trace = res.instructions_and_trace
        if res.exec_time_ns is not None:
            print(f"HW exec time: {res.exec_time_ns} ns")
    if bool(int(_os.environ.get("BASS_GAT_DEBUG", "0"))):
        kernel.debug_results = res.results
    return res.results[0]["y"]



# revision 6
# speedup vs baseline: 2.8307x; 2.8307x over previous
import sys
import numpy as np

sys.path.insert(0, "/opt/trn_rl_repo")
sys.path.insert(0, "/opt/trn_rl_repo/concourse")

import ml_dtypes
import concourse.bass as bass
import concourse.bacc as bacc
import concourse.mybir as mybir
import concourse.tile as tile
from concourse.bass import IndirectOffsetOnAxis
from concourse.bass_utils import run_bass_kernel_spmd
from concourse.masks import make_identity

F32 = mybir.dt.float32
BF16 = mybir.dt.bfloat16
I32 = mybir.dt.int32
BFNP = ml_dtypes.bfloat16

N = 20000
E = 160000
B = 128
NDEV = 8
NPD = N // NDEV          # 2500 nodes per device
NT = (NPD + 127) // 128  # 20 dst tiles per device
GT = 4                   # tiles per allgather group
NG = NT // GT            # 5 groups
H = 4
C1IN, C1 = 768, 512
C2IN, C2 = 512, 256
CC1 = C1IN // 128        # 6
CC2 = C2IN // 128        # 4
HC1 = H * C1             # 2048
HC2 = H * C2             # 1024
XAW = 800                # [x 768 | as1 4 | ad1 4 | pad] bf16 row (1600B)
TW2 = 1056               # [xw2 1024 | as2 4 | ad2 4 | pad] bf16 row (2112B)
W2C = HC2 + 8            # 1032 cols of [W2 | wa2]
NEG = 0.2


def _host_prep(edge_index, batch):
    """Integer-only preprocessing: edge partitioning, sorting, chunk layout."""
    src = np.concatenate([edge_index[0], np.arange(N, dtype=np.int64)]).astype(np.int64)
    dst = np.concatenate([edge_index[1], np.arange(N, dtype=np.int64)]).astype(np.int64)
    order = np.argsort(dst, kind="stable")
    src, dst = src[order], dst[order]

    dev = dst // NPD
    tloc = (dst % NPD) // 128
    cnt = np.zeros((NDEV, NT), dtype=np.int64)
    for d in range(NDEV):
        m = dev == d
        cnt[d] = np.bincount(tloc[m], minlength=NT)
    Ks = [max(1, int(np.ceil(cnt[:, t].max() / 128.0))) for t in range(NT)]
    SK = sum(Ks)
    offs = np.cumsum([0] + Ks)

    # t2_full row index for source node s (grouped allgather layout)
    s_dev = src // NPD
    s_loc = src % NPD
    s_tl = s_loc // 128
    s_r = s_loc % 128
    t2row = (s_tl // GT) * (NDEV * GT * 128) + s_dev * (GT * 128) + (s_tl % GT) * 128 + s_r

    xidx = np.zeros((NDEV, 128, SK), dtype=np.int32)   # into xa rows
    x2idx = np.zeros((NDEV, 128, SK), dtype=np.int32)  # into t2_full rows
    dstf = np.full((NDEV, 128, SK), -1.0, dtype=np.float32)
    dstfR = np.full((NDEV, SK, 128), -1.0, dtype=np.float32)

    for d in range(NDEV):
        m = dev == d
        s_d, t_d, dl_d, r2_d = src[m], tloc[m], (dst[m] % NPD) % 128, t2row[m]
        for t in range(NT):
            mt = t_d == t
            s_t = s_d[mt]
            dl_t = dl_d[mt]
            r2_t = r2_d[mt]
            o = offs[t]
            j = np.arange(len(s_t))
            xidx[d, j % 128, o + j // 128] = s_t
            x2idx[d, j % 128, o + j // 128] = r2_t
            dstf[d, j % 128, o + j // 128] = dl_t.astype(np.float32)
            dstfR[d, o + j // 128, j % 128] = dl_t.astype(np.float32)

    batchf = np.full((NDEV, 128, NT), -1.0, dtype=np.float32)
    b_np = np.asarray(batch).astype(np.int64)
    for d in range(NDEV):
        for t in range(NT):
            rows = min(128, NPD - t * 128)
            g = b_np[d * NPD + t * 128: d * NPD + t * 128 + rows]
            batchf[d, :rows, t] = g.astype(np.float32)

    return Ks, offs, SK, xidx, x2idx, dstf, dstfR, batchf


def _build(Ks, offs, SK):
    """Emit the Bass program (identical for all 8 cores)."""
    nc = bacc.Bacc("TRN2", target_bir_lowering=False, debug=False, num_devices=NDEV)

    # ---- I/O ----
    xa_t = nc.dram_tensor("xa", [N, XAW], BF16, kind="ExternalInput")
    xlocT_t = nc.dram_tensor("xlocT", [C1IN, NPD], BF16, kind="ExternalInput")
    W1_t = nc.dram_tensor("W1", [C1IN, HC1], BF16, kind="ExternalInput")
    W2c_t = nc.dram_tensor("W2c", [C2IN, W2C], BF16, kind="ExternalInput")
    wa1_t = nc.dram_tensor("wa1", [C1IN, 8], BF16, kind="ExternalInput")
    b1_t = nc.dram_tensor("b1", [C1], BF16, kind="ExternalInput")
    b2_t = nc.dram_tensor("b2", [C2], F32, kind="ExternalInput")
    fcW_t = nc.dram_tensor("fcW", [C2, 2], F32, kind="ExternalInput")
    fcb_t = nc.dram_tensor("fcb", [2], F32, kind="ExternalInput")
    xidx_t = nc.dram_tensor("xidx", [128, SK], I32, kind="ExternalInput")
    x2idx_t = nc.dram_tensor("x2idx", [128, SK], I32, kind="ExternalInput")
    dstf_t = nc.dram_tensor("dstf", [128, SK], F32, kind="ExternalInput")
    dstfR_t = nc.dram_tensor("dstfR", [SK, 128], F32, kind="ExternalInput")
    batchf_t = nc.dram_tensor("batchf", [128, NT], F32, kind="ExternalInput")
    y_t = nc.dram_tensor("y", [B, 2], F32, kind="ExternalOutput")

    # ---- internal DRAM ----
    asad1_loc = nc.dram_tensor("asad1_loc", [NPD, 8], BF16)
    asad1_full = nc.dram_tensor("asad1_full", [N, 8], BF16, addr_space="Shared")
    t2_loc = nc.dram_tensor("t2_loc", [NT * 128, TW2], BF16)
    t2_full = nc.dram_tensor("t2_full", [NG * NDEV * GT * 128, TW2], BF16,
                             addr_space="Shared")
    rdscr = nc.dram_tensor("rdscr", [NT, 512], F32)
    pc_loc = nc.dram_tensor("pc_loc", [B, C2 + 1], F32)
    pc_red = nc.dram_tensor("pc_red", [B, C2 + 1], F32, addr_space="Shared")

    RG = [list(range(NDEV))]
    KMAX = max(Ks)

    import os as _os
    DEBUG = bool(int(_os.environ.get("BASS_GAT_DEBUG", "0")))
    if DEBUG:
        dbg_h1 = nc.dram_tensor("dbg_h1", [NT * 128, C1], F32, kind="ExternalOutput")
        dbg_t2 = nc.dram_tensor("dbg_t2", [NG * NDEV * GT * 128, TW2], F32,
                                kind="ExternalOutput")
        dbg_pc = nc.dram_tensor("dbg_pc", [B, C2 + 1], F32, kind="ExternalOutput")

    with tile.TileContext(nc) as tc:
        with (
            tc.tile_pool(name="const", bufs=1) as cp,
            tc.tile_pool(name="small", bufs=2) as sp,
            tc.tile_pool(name="selp", bufs=2) as selp,
            tc.tile_pool(name="selwp", bufs=3) as selwp,
            tc.tile_pool(name="xgp", bufs=2) as xgp,
            tc.tile_pool(name="utp", bufs=2) as utp,
            tc.tile_pool(name="t2p", bufs=2) as t2p,
        ):
            # ================= constants =================
            ident = cp.tile([128, 128], F32, tag="ident")
            make_identity(nc, ident[:])
            identb = cp.tile([128, 128], BF16, tag="identb")
            make_identity(nc, identb[:])
            iota_i = cp.tile([128, 128], I32, tag="iota_i")
            nc.gpsimd.iota(iota_i[:], pattern=[[1, 128]], base=0, channel_multiplier=0)
            iotaT = cp.tile([128, 128], F32, tag="iotaT")
            nc.vector.tensor_copy(out=iotaT[:], in_=iota_i[:])
            iota_ci = cp.tile([128, 1], I32, tag="iota_ci")
            nc.gpsimd.iota(iota_ci[:], pattern=[[1, 1]], base=0, channel_multiplier=1)
            iotaC = cp.tile([128, 1], F32, tag="iotaC")
            nc.vector.tensor_copy(out=iotaC[:], in_=iota_ci[:])
            ones1 = cp.tile([1, 128], BF16, tag="ones1")
            nc.vector.memset(ones1[:], 1.0)
            ones1f = cp.tile([1, 128], F32, tag="ones1f")
            nc.vector.memset(ones1f[:], 1.0)

            b1_sb = cp.tile([1, C1], BF16, tag="b1")
            nc.sync.dma_start(out=b1_sb[:], in_=b1_t[None, :])
            b2bc = cp.tile([128, C2], F32, tag="b2bc")
            nc.sync.dma_start(out=b2bc[:], in_=b2_t[None, :].to_broadcast([128, C2]))
            fcb_sb = cp.tile([1, 2], F32, tag="fcb")
            nc.sync.dma_start(out=fcb_sb[:], in_=fcb_t[None, :])
            fcW_sb = cp.tile([128, 4], F32, tag="fcW")
            for c in range(2):
                nc.sync.dma_start(out=fcW_sb[:, 2 * c:2 * c + 2],
                                  in_=fcW_t[c * 128:(c + 1) * 128, :])

            W1_sb = cp.tile([128, CC1 * HC1], BF16, tag="W1")
            for c in range(CC1):
                nc.sync.dma_start(out=W1_sb[:, c * HC1:(c + 1) * HC1],
                                  in_=W1_t[c * 128:(c + 1) * 128, :])
            W2c_sb = cp.tile([128, CC2 * W2C], BF16, tag="W2c")
            for c in range(CC2):
                nc.sync.dma_start(out=W2c_sb[:, c * W2C:(c + 1) * W2C],
                                  in_=W2c_t[c * 128:(c + 1) * 128, :])
            wa1_sb = cp.tile([128, CC1 * 8], BF16, tag="wa1")
            nc.sync.dma_start(
                out=wa1_sb[:].rearrange("p (c j) -> p c j", j=8),
                in_=wa1_t[:].rearrange("(c p) j -> p c j", p=128))
            batchf_sb = cp.tile([128, NT], F32, tag="batchf")
            nc.sync.dma_start(out=batchf_sb[:], in_=batchf_t[:, :])
            poolacc = cp.tile([128, C2 + 1], F32, tag="poolacc")
            nc.vector.memset(poolacc[:], 0.0)

            # ============ asad1 = x_loc @ wa1 ============
            with (
                tc.tile_pool(name="prep", bufs=2) as pp,
                tc.tile_pool(name="pshp", bufs=2, space="PSUM") as pshp,
            ):
                for t in range(NT):
                    rows = min(128, NPD - t * 128)
                    xT = pp.tile([128, CC1 * 128], BF16, tag="xT")
                    nc.sync.dma_start(
                        out=xT[:, :CC1 * rows].rearrange("p (c n) -> p c n", c=CC1),
                        in_=xlocT_t[:, t * 128: t * 128 + rows].rearrange(
                            "(c p) n -> p c n", p=128))
                    ps = pshp.tile([128, 8], F32, tag="ps")
                    for c in range(CC1):
                        nc.tensor.matmul(out=ps[:rows, :],
                                         lhsT=xT[:, c * rows:(c + 1) * rows],
                                         rhs=wa1_sb[:, c * 8:(c + 1) * 8],
                                         start=(c == 0), stop=(c == CC1 - 1))
                    as1 = pp.tile([128, 8], BF16, tag="as1")
                    nc.vector.tensor_copy(out=as1[:rows, :], in_=ps[:rows, :])
                    nc.sync.dma_start(out=asad1_loc[t * 128: t * 128 + rows, :],
                                      in_=as1[:rows, :])

            nc.gpsimd.collective_compute(
                "AllGather", mybir.AluOpType.bypass, replica_groups=RG,
                ins=[asad1_loc[:, :]], outs=[asad1_full[:, :]])
            nc.sync.dma_start(out=xa_t[:, C1IN:C1IN + 8], in_=asad1_full[:, :])

            # ================= layer 1 sweep =================
            with (
                tc.tile_pool(name="psu", bufs=1, space="PSUM") as psu,
                tc.tile_pool(name="psh", bufs=1, space="PSUM") as psh,
                tc.tile_pool(name="psr", bufs=1, space="PSUM") as psr,
            ):
                for t in range(NT):
                    K = Ks[t]
                    o = offs[t]
                    rows = min(128, NPD - t * 128)

                    idx_sb = sp.tile([128, KMAX], I32, tag="idx")
                    nc.sync.dma_start(out=idx_sb[:, :K], in_=xidx_t[:, o:o + K])
                    dstf_sb = sp.tile([128, KMAX], F32, tag="dstf")
                    nc.sync.dma_start(out=dstf_sb[:, :K], in_=dstf_t[:, o:o + K])
                    ad_sb = sp.tile([128, 4], BF16, tag="ad")
                    if rows < 128:
                        nc.vector.memset(ad_sb[:], 0.0)
                    nc.sync.dma_start(out=ad_sb[:rows, :],
                                      in_=asad1_loc[t * 128:t * 128 + rows, 4:8])

                    # ---------- gather x rows ----------
                    xg = xgp.tile([128, KMAX * XAW], BF16, tag="xg")
                    for k in range(K):
                        nc.gpsimd.indirect_dma_start(
                            out=xg[:, k * XAW:(k + 1) * XAW], out_offset=None,
                            in_=xa_t[:, :],
                            in_offset=IndirectOffsetOnAxis(
                                ap=idx_sb[:, k:k + 1], axis=0))
                    xg3 = xg[:, :K * XAW].rearrange("p (k d) -> p k d", d=XAW)

                    # ---------- selection matrices (batched) ----------
                    dstfT = sp.tile([128, KMAX * 128], F32, tag="dstfT")
                    nc.sync.dma_start(
                        out=dstfT[:, :K * 128].rearrange("p (k e) -> p k e", e=128),
                        in_=dstfR_t[None, o:o + K, :].to_broadcast([128, K, 128]))
                    selT = selp.tile([128, KMAX * 128], BF16, tag="selT")
                    nc.vector.tensor_tensor(
                        out=selT[:, :K * 128].rearrange("p (k e) -> p k e", e=128),
                        in0=iotaC[:, :, None].to_broadcast([128, K, 128]),
                        in1=dstfT[:, :K * 128].rearrange("p (k e) -> p k e", e=128),
                        op=mybir.AluOpType.is_equal)
                    sel = selp.tile([128, KMAX * 128], BF16, tag="sel")
                    nc.vector.tensor_tensor(
                        out=sel[:, :K * 128].rearrange("p (k d) -> p k d", d=128),
                        in0=dstf_sb[:, :K, None].to_broadcast([128, K, 128]),
                        in1=iotaT[:, None, :].to_broadcast([128, K, 128]),
                        op=mybir.AluOpType.is_equal)

                    # ---------- e = lrelu(as+ad); ex = exp ----------
                    ade_ps = psh.tile([128, 4 * KMAX], F32, tag="hold")
                    for k in range(K):
                        nc.tensor.matmul(out=ade_ps[:, 4 * k:4 * k + 4],
                                         lhsT=selT[:, k * 128:(k + 1) * 128],
                                         rhs=ad_sb[:],
                                         start=(k == 0), stop=(k == K - 1))
                    asg = sp.tile([128, 4 * KMAX], F32, tag="asg")
                    nc.vector.tensor_copy(out=asg[:, :4 * K].rearrange(
                        "p (k s) -> p k s", s=4),
                        in_=xg3[:, :, C1IN:C1IN + 4])
                    z = sp.tile([128, 4 * KMAX], F32, tag="z")
                    nc.vector.tensor_tensor(out=z[:, :4 * K], in0=ade_ps[:, :4 * K],
                                            in1=asg[:, :4 * K],
                                            op=mybir.AluOpType.add)
                    zs = sp.tile([128, 4 * KMAX], F32, tag="zs")
                    nc.vector.tensor_scalar_mul(zs[:, :4 * K], z[:, :4 * K], NEG)
                    nc.vector.tensor_tensor(out=z[:, :4 * K], in0=z[:, :4 * K],
                                            in1=zs[:, :4 * K],
                                            op=mybir.AluOpType.max)
                    ex = sp.tile([128, 4 * KMAX], BF16, tag="ex")
                    nc.scalar.activation(out=ex[:, :4 * K], in_=z[:, :4 * K],
                                         func=mybir.ActivationFunctionType.Exp)

                    # ---------- denT[h, d] = sum_e ex[e,h] sel[e,d] ----------
                    denT_ps = psh.tile([4, 128], F32, tag="hold")
                    for k in range(K):
                        nc.tensor.matmul(out=denT_ps[:],
                                         lhsT=ex[:, 4 * k:4 * k + 4],
                                         rhs=sel[:, k * 128:(k + 1) * 128],
                                         start=(k == 0), stop=(k == K - 1))

                    # ---------- aggregate ut[f,(h,d)] += x ex ----------
                    ut_ps = psu.tile([128, CC1 * 512], F32, tag="ut")
                    for k in range(K):
                        selw = selwp.tile([128, 512], BF16, tag="selw")
                        nc.vector.tensor_tensor(
                            out=selw[:].rearrange("p (h d) -> p h d", d=128),
                            in0=sel[:, k * 128:(k + 1) * 128][:, None, :]
                                .to_broadcast([128, 4, 128]),
                            in1=ex[:, 4 * k:4 * k + 4][:, :, None]
                                .to_broadcast([128, 4, 128]),
                            op=mybir.AluOpType.mult)
                        for c in range(CC1):
                            nc.tensor.matmul(
                                out=ut_ps[:, c * 512:(c + 1) * 512],
                                lhsT=xg[:, k * XAW + c * 128: k * XAW + (c + 1) * 128],
                                rhs=selw[:],
                                start=(k == 0), stop=(k == K - 1))

                    # ---------- rdenT broadcast + divide ----------
                    rdT = sp.tile([4, 128], F32, tag="rdT")
                    nc.vector.tensor_scalar(out=rdT[:], in0=denT_ps[:],
                                            scalar1=4.0, scalar2=1e-16,
                                            op0=mybir.AluOpType.mult,
                                            op1=mybir.AluOpType.max)
                    nc.vector.reciprocal(out=rdT[:], in_=rdT[:])
                    nc.sync.dma_start(
                        out=rdscr[t:t + 1, :].rearrange("a (p f) -> (a p) f", p=4),
                        in_=rdT[:])
                    rdb = sp.tile([128, 512], F32, tag="rdb")
                    nc.sync.dma_start(out=rdb[:],
                                      in_=rdscr[t:t + 1, :].to_broadcast([128, 512]))
                    ut_sb = utp.tile([128, CC1 * 512], BF16, tag="ut")
                    for c in range(CC1):
                        nc.vector.tensor_tensor(out=ut_sb[:, c * 512:(c + 1) * 512],
                                                in0=ut_ps[:, c * 512:(c + 1) * 512],
                                                in1=rdb[:],
                                                op=mybir.AluOpType.mult)

                    # ---------- project + bias ----------
                    out_ps = psh.tile([128, C1], F32, tag="hold")
                    first = True
                    for h in range(H):
                        for c in range(CC1):
                            nc.tensor.matmul(
                                out=out_ps[:],
                                lhsT=ut_sb[:, c * 512 + h * 128: c * 512 + (h + 1) * 128],
                                rhs=W1_sb[:, c * HC1 + h * C1: c * HC1 + (h + 1) * C1],
                                start=first, stop=False)
                            first = False
                    nc.tensor.matmul(out=out_ps[:], lhsT=ones1[:], rhs=b1_sb[:],
                                     start=False, stop=True)
                    h1_sb = t2p.tile([128, C1], BF16, tag="h1")
                    nc.vector.tensor_copy(out=h1_sb[:], in_=out_ps[:])
                    if DEBUG:
                        h1f = sp.tile([128, C1], F32, tag="h1f")
                        nc.vector.tensor_copy(out=h1f[:], in_=out_ps[:])
                        nc.sync.dma_start(out=dbg_h1[t * 128:(t + 1) * 128, :],
                                          in_=h1f[:])

                    # ---------- xw2 = h1 @ [W2 | wa2] ----------
                    h1T = t2p.tile([128, C1], BF16, tag="h1T")
                    for c in range(CC2):
                        tp = psr.tile([128, 128], BF16, tag="rot")
                        nc.tensor.transpose(out=tp[:],
                                            in_=h1_sb[:, c * 128:(c + 1) * 128],
                                            identity=identb[:])
                        nc.vector.tensor_copy(out=h1T[:, c * 128:(c + 1) * 128],
                                              in_=tp[:])
                    xw2_sb = t2p.tile([128, TW2], BF16, tag="xw2")
                    for (lo, w) in ((0, 512), (512, 512), (1024, 8)):
                        xp = psr.tile([128, 512], F32, tag="rot")
                        for c in range(CC2):
                            nc.tensor.matmul(
                                out=xp[:, :w],
                                lhsT=h1T[:, c * 128:(c + 1) * 128],
                                rhs=W2c_sb[:, c * W2C + lo: c * W2C + lo + w],
                                start=(c == 0), stop=(c == CC2 - 1))
                        nc.vector.tensor_copy(out=xw2_sb[:, lo:lo + w], in_=xp[:, :w])
                    nc.sync.dma_start(out=t2_loc[t * 128:(t + 1) * 128, :],
                                      in_=xw2_sb[:])

                    if t % GT == GT - 1:
                        g = t // GT
                        nc.gpsimd.collective_compute(
                            "AllGather", mybir.AluOpType.bypass, replica_groups=RG,
                            ins=[t2_loc[g * GT * 128:(g + 1) * GT * 128, :]],
                            outs=[t2_full[g * NDEV * GT * 128:
                                          (g + 1) * NDEV * GT * 128, :]])

            if DEBUG:
                t2f = sp.tile([128, TW2], F32, tag="t2f")
                for i in range(NG * NDEV * GT):
                    t2b = sp.tile([128, TW2], BF16, tag="t2b")
                    nc.sync.dma_start(out=t2b[:],
                                      in_=t2_full[i * 128:(i + 1) * 128, :])
                    nc.vector.tensor_copy(out=t2f[:], in_=t2b[:])
                    nc.sync.dma_start(out=dbg_t2[i * 128:(i + 1) * 128, :],
                                      in_=t2f[:])

            # ================= layer 2 sweep + pooling =================
            with (
                tc.tile_pool(name="psu2", bufs=2, space="PSUM") as psu2,
                tc.tile_pool(name="psh2", bufs=2, space="PSUM") as psh2,
            ):
                for t in range(NT):
                    K = Ks[t]
                    o = offs[t]
                    rows = min(128, NPD - t * 128)

                    idx_sb = sp.tile([128, KMAX], I32, tag="idx")
                    nc.sync.dma_start(out=idx_sb[:, :K], in_=x2idx_t[:, o:o + K])
                    dstf_sb = sp.tile([128, KMAX], F32, tag="dstf")
                    nc.sync.dma_start(out=dstf_sb[:, :K], in_=dstf_t[:, o:o + K])
                    ad_sb = sp.tile([128, 4], BF16, tag="ad")
                    if rows < 128:
                        nc.vector.memset(ad_sb[:], 0.0)
                    nc.sync.dma_start(out=ad_sb[:rows, :],
                                      in_=t2_loc[t * 128:t * 128 + rows,
                                                 HC2 + 4:HC2 + 8])

                    hg = xgp.tile([128, KMAX * TW2], BF16, tag="hg")
                    for k in range(K):
                        nc.gpsimd.indirect_dma_start(
                            out=hg[:, k * TW2:(k + 1) * TW2], out_offset=None,
                            in_=t2_full[:, :],
                            in_offset=IndirectOffsetOnAxis(
                                ap=idx_sb[:, k:k + 1], axis=0))
                    hg3 = hg[:, :K * TW2].rearrange("p (k d) -> p k d", d=TW2)

                    dstfT = sp.tile([128, KMAX * 128], F32, tag="dstfT")
                    nc.sync.dma_start(
                        out=dstfT[:, :K * 128].rearrange("p (k e) -> p k e", e=128),
                        in_=dstfR_t[None, o:o + K, :].to_broadcast([128, K, 128]))
                    selT = selp.tile([128, KMAX * 128], BF16, tag="selT")
                    nc.vector.tensor_tensor(
                        out=selT[:, :K * 128].rearrange("p (k e) -> p k e", e=128),
                        in0=iotaC[:, :, None].to_broadcast([128, K, 128]),
                        in1=dstfT[:, :K * 128].rearrange("p (k e) -> p k e", e=128),
                        op=mybir.AluOpType.is_equal)
                    sel = selp.tile([128, KMAX * 128], BF16, tag="sel")
                    nc.vector.tensor_tensor(
                        out=sel[:, :K * 128].rearrange("p (k d) -> p k d", d=128),
                        in0=dstf_sb[:, :K, None].to_broadcast([128, K, 128]),
                        in1=iotaT[:, None, :].to_broadcast([128, K, 128]),
                        op=mybir.AluOpType.is_equal)

                    ade_ps = psh2.tile([128, 4 * KMAX], F32, tag="hold")
                    for k in range(K):
                        nc.tensor.matmul(out=ade_ps[:, 4 * k:4 * k + 4],
                                         lhsT=selT[:, k * 128:(k + 1) * 128],
                                         rhs=ad_sb[:],
                                         start=(k == 0), stop=(k == K - 1))
                    asg = sp.tile([128, 4 * KMAX], F32, tag="asg")
                    nc.vector.tensor_copy(out=asg[:, :4 * K].rearrange(
                        "p (k s) -> p k s", s=4),
                        in_=hg3[:, :, HC2:HC2 + 4])
                    z = sp.tile([128, 4 * KMAX], F32, tag="z")
                    nc.vector.tensor_tensor(out=z[:, :4 * K], in0=ade_ps[:, :4 * K],
                                            in1=asg[:, :4 * K],
                                            op=mybir.AluOpType.add)
                    zs = sp.tile([128, 4 * KMAX], F32, tag="zs")
                    nc.vector.tensor_scalar_mul(zs[:, :4 * K], z[:, :4 * K], NEG)
                    nc.vector.tensor_tensor(out=z[:, :4 * K], in0=z[:, :4 * K],
                                            in1=zs[:, :4 * K],
                                            op=mybir.AluOpType.max)
                    ex = sp.tile([128, 4 * KMAX], BF16, tag="ex")
                    nc.scalar.activation(out=ex[:, :4 * K], in_=z[:, :4 * K],
                                         func=mybir.ActivationFunctionType.Exp)

                    # agg[d, (h,c)] += ex * xw2 ; den[d, h] += ex
                    agg_ps = psu2.tile([128, HC2], F32, tag="ut2")
                    den_ps = psh2.tile([128, 4], F32, tag="hold")
                    for k in range(K):
                        exw = selwp.tile([128, HC2], BF16, tag="exw")
                        nc.vector.tensor_tensor(
                            out=exw[:].rearrange("p (h c) -> p h c", c=C2),
                            in0=hg3[:, k, :HC2].rearrange("p (h c) -> p h c", c=C2),
                            in1=ex[:, 4 * k:4 * k + 4][:, :, None]
                                .to_broadcast([128, 4, C2]),
                            op=mybir.AluOpType.mult)
                        nc.tensor.matmul(out=agg_ps[:, 0:512],
                                         lhsT=sel[:, k * 128:(k + 1) * 128],
                                         rhs=exw[:, 0:512],
                                         start=(k == 0), stop=(k == K - 1))
                        nc.tensor.matmul(out=agg_ps[:, 512:1024],
                                         lhsT=sel[:, k * 128:(k + 1) * 128],
                                         rhs=exw[:, 512:1024],
                                         start=(k == 0), stop=(k == K - 1))
                        nc.tensor.matmul(out=den_ps[:],
                                         lhsT=sel[:, k * 128:(k + 1) * 128],
                                         rhs=ex[:, 4 * k:4 * k + 4],
                                         start=(k == 0), stop=(k == K - 1))

                    rd2 = sp.tile([128, 4], F32, tag="rd2")
                    nc.vector.tensor_scalar(out=rd2[:], in0=den_ps[:],
                                            scalar1=4.0, scalar2=1e-16,
                                            op0=mybir.AluOpType.mult,
                                            op1=mybir.AluOpType.max)
                    nc.vector.reciprocal(out=rd2[:], in_=rd2[:])
                    h2acc = sp.tile([128, C2], F32, tag="h2acc")
                    nc.vector.tensor_scalar_mul(h2acc[:], agg_ps[:, 0:C2],
                                                rd2[:, 0:1])
                    for h in range(1, H):
                        nc.vector.scalar_tensor_tensor(
                            out=h2acc[:], in0=agg_ps[:, h * C2:(h + 1) * C2],
                            scalar=rd2[:, h:h + 1], in1=h2acc[:],
                            op0=mybir.AluOpType.mult, op1=mybir.AluOpType.add)
                    h2p = t2p.tile([128, C2 + 1], BF16, tag="h2p")
                    nc.vector.tensor_tensor(out=h2p[:, :C2], in0=h2acc[:],
                                            in1=b2bc[:], op=mybir.AluOpType.add)
                    nc.vector.memset(h2p[:, C2:C2 + 1], 1.0)

                    selB = sp.tile([128, 128], BF16, tag="selB")
                    nc.vector.tensor_tensor(
                        out=selB[:],
                        in0=batchf_sb[:, t:t + 1].to_broadcast([128, 128]),
                        in1=iotaT[:], op=mybir.AluOpType.is_equal)
                    pc_ps = psh2.tile([128, C2 + 1], F32, tag="hold")
                    nc.tensor.matmul(out=pc_ps[:], lhsT=selB[:], rhs=h2p[:],
                                     start=True, stop=True)
                    nc.vector.tensor_tensor(out=poolacc[:], in0=poolacc[:],
                                            in1=pc_ps[:], op=mybir.AluOpType.add)

                # ================= pool reduce + FC =================
                nc.sync.dma_start(out=pc_loc[:, :], in_=poolacc[:])
                nc.gpsimd.collective_compute(
                    "AllReduce", mybir.AluOpType.add, replica_groups=RG,
                    ins=[pc_loc[:, :]], outs=[pc_red[:, :]])
                pc_sb = sp.tile([128, C2 + 1], F32, tag="pc")
                nc.sync.dma_start(out=pc_sb[:], in_=pc_red[:, :])
                if DEBUG:
                    nc.sync.dma_start(out=dbg_pc[:, :], in_=pc_red[:, :])
                cnt = sp.tile([128, 1], F32, tag="cnt")
                nc.vector.tensor_scalar_max(cnt[:], pc_sb[:, C2:C2 + 1], 1.0)
                nc.vector.reciprocal(out=cnt[:], in_=cnt[:])
                g_sb = sp.tile([128, C2], F32, tag="g")
                nc.vector.tensor_scalar_mul(g_sb[:], pc_sb[:, :C2], cnt[:, :1])

                y_ps = psh2.tile([128, 2], F32, tag="hold")
                for c in range(2):
                    tp = psu2.tile([128, 128], F32, tag="ut2")
                    nc.tensor.transpose(out=tp[:], in_=g_sb[:, c * 128:(c + 1) * 128],
                                        identity=ident[:])
                    gT = sp.tile([128, 128], F32, tag="gT")
                    nc.vector.tensor_copy(out=gT[:], in_=tp[:])
                    nc.tensor.matmul(out=y_ps[:], lhsT=gT[:],
                                     rhs=fcW_sb[:, 2 * c:2 * c + 2],
                                     start=(c == 0), stop=False)
                nc.tensor.matmul(out=y_ps[:], lhsT=ones1f[:], rhs=fcb_sb[:],
                                 start=False, stop=True)
                y_sb = sp.tile([128, 2], F32, tag="y")
                nc.vector.tensor_copy(out=y_sb[:], in_=y_ps[:])
                nc.sync.dma_start(out=y_t[:, :], in_=y_sb[:])

    nc.compile()
    return nc


_CACHE = {}


def kernel(**inputs):
    x = np.ascontiguousarray(np.asarray(inputs["x"], dtype=np.float32))
    edge_index = np.asarray(inputs["edge_index"])
    batch = np.asarray(inputs["batch"])
    W1 = np.asarray(inputs["W1"], dtype=np.float32)
    W2 = np.asarray(inputs["W2"], dtype=np.float32)
    a_src1 = np.asarray(inputs["a_src1"], dtype=np.float32)
    a_dst1 = np.asarray(inputs["a_dst1"], dtype=np.float32)
    a_src2 = np.asarray(inputs["a_src2"], dtype=np.float32)
    a_dst2 = np.asarray(inputs["a_dst2"], dtype=np.float32)
    b1 = np.asarray(inputs["b1"], dtype=np.float32)
    b2 = np.asarray(inputs["b2"], dtype=np.float32)
    fcW = np.ascontiguousarray(np.asarray(inputs["fcW"], dtype=np.float32))
    fcb = np.asarray(inputs["fcb"], dtype=np.float32)

    Ks, offs, SK, xidx, x2idx, dstf, dstfR, batchf = _host_prep(edge_index, batch)

    key = (tuple(Ks),)
    if key not in _CACHE:
        _CACHE[key] = _build(Ks, offs, SK)
    nc = _CACHE[key]

    # weight-only prep: wa = W^T a per head (folded attention projections)
    wa1 = np.zeros((C1IN, 8), dtype=np.float32)
    wa2 = np.zeros((C2IN, 8), dtype=np.float32)
    for h in range(H):
        wa1[:, h] = W1[:, h * C1:(h + 1) * C1] @ a_src1[h]
        wa1[:, 4 + h] = W1[:, h * C1:(h + 1) * C1] @ a_dst1[h]
        wa2[:, h] = W2[:, h * C2:(h + 1) * C2] @ a_src2[h]
        wa2[:, 4 + h] = W2[:, h * C2:(h + 1) * C2] @ a_dst2[h]
    W2c = np.concatenate([W2, wa2], axis=1)

    xa = np.zeros((N, XAW), dtype=BFNP)
    xa[:, :C1IN] = x.astype(BFNP)

    in_maps = []
    for d in range(NDEV):
        xloc = x[d * NPD:(d + 1) * NPD]
        in_maps.append({
            "xa": xa,
            "xlocT": np.ascontiguousarray(xloc.T).astype(BFNP),
            "W1": W1.astype(BFNP), "W2c": W2c.astype(BFNP),
            "wa1": wa1.astype(BFNP), "b1": b1.astype(BFNP), "b2": b2,
            "fcW": fcW, "fcb": fcb,
            "xidx": xidx[d], "x2idx": x2idx[d], "dstf": dstf[d],
            "dstfR": dstfR[d], "batchf": batchf[d],
        })

    import os as _os
    trace = bool(int(_os.environ.get("BASS_GAT_TRACE", "0")))
    kwargs = {}
    if trace:
        _setup_ntff_hook()
        kwargs = dict(trace=True, trace_cores=[0])
    res = run_bass_kernel_spmd(nc, in_maps, core_ids=list(range(NDEV)), **kwargs)
    if trace:
        kernel.last_exec_ns = res.exec_time_ns
        kernel.last_trace = res.instructions_and_trace
        if res.exec_time_ns is not None:
            print(f"HW exec time: {res.exec_time_ns} ns")
    if bool(int(_os.environ.get("BASS_GAT_DEBUG", "0"))):
        kernel.debug_results = res.results
    return res.results[0]["y"]


def _setup_ntff_hook():
    """The image's antenv lacks axon_hooks; synthesize it and register the
    ctypes NTFF profiling hook so trace=True works."""
    import types
    import antenv
    if hasattr(antenv, "axon_hooks"):
        return
    mod = types.ModuleType("antenv.axon_hooks")
    state = {"hook": None}
    mod.set_axon_ntff_profile_hook = lambda h: state.__setitem__("hook", h)
    mod.get_axon_ntff_profile_hook = lambda: state["hook"]
    sys.modules["antenv.axon_hooks"] = mod
    antenv.axon_hooks = mod
    try:
        from trn_agent_boot.trn_boot import _ntff_profile_via_ctypes
        hook = _ntff_profile_via_ctypes("/opt/axon/libaxon_pjrt.so")
        mod.set_axon_ntff_profile_hook(hook)
    except Exception as e:
        print("ntff hook setup failed:", e)


# revision 11
# speedup vs baseline: 3.0163x; 1.0656x over previous
import sys
import numpy as np

sys.path.insert(0, "/opt/trn_rl_repo")
sys.path.insert(0, "/opt/trn_rl_repo/concourse")

import ml_dtypes
import concourse.bass as bass
import concourse.bacc as bacc
import concourse.mybir as mybir
import concourse.tile as tile
from concourse.bass import IndirectOffsetOnAxis
from concourse.bass_utils import run_bass_kernel_spmd
from concourse.masks import make_identity

F32 = mybir.dt.float32
BF16 = mybir.dt.bfloat16
I32 = mybir.dt.int32
BFNP = ml_dtypes.bfloat16

N = 20000
E = 160000
B = 128
NDEV = 8
NPD = N // NDEV          # 2500 nodes per device
NT = (NPD + 127) // 128  # 20 dst tiles per device
GT = 4                   # tiles per allgather group
NG = NT // GT            # 5 groups
H = 4
C1IN, C1 = 768, 512
C2IN, C2 = 512, 256
CC1 = C1IN // 128        # 6
CC2 = C2IN // 128        # 4
HC1 = H * C1             # 2048
HC2 = H * C2             # 1024
XAW = 768                # x row bf16 (1536B)
TW2 = 1056               # [xw2 1024 | as2 4 | ad2 4 | pad] bf16 row (2112B)
W2C = HC2 + 8            # 1032 cols of [W2 | wa2]
NEG = 0.2


def _host_prep(edge_index, batch):
    """Integer-only preprocessing: edge partitioning, sorting, chunk layout."""
    src = np.concatenate([edge_index[0], np.arange(N, dtype=np.int64)]).astype(np.int64)
    dst = np.concatenate([edge_index[1], np.arange(N, dtype=np.int64)]).astype(np.int64)
    order = np.argsort(dst, kind="stable")
    src, dst = src[order], dst[order]

    dev = dst // NPD
    tloc = (dst % NPD) // 128
    cnt = np.zeros((NDEV, NT), dtype=np.int64)
    for d in range(NDEV):
        m = dev == d
        cnt[d] = np.bincount(tloc[m], minlength=NT)
    Ks = [max(1, int(np.ceil(cnt[:, t].max() / 128.0))) for t in range(NT)]
    SK = sum(Ks)
    offs = np.cumsum([0] + Ks)

    # t2_full row index for source node s (grouped allgather layout)
    s_dev = src // NPD
    s_loc = src % NPD
    s_tl = s_loc // 128
    s_r = s_loc % 128
    t2row = (s_tl // GT) * (NDEV * GT * 128) + s_dev * (GT * 128) + (s_tl % GT) * 128 + s_r

    xidx = np.zeros((NDEV, 128, SK), dtype=np.int32)   # into xa rows
    x2idx = np.zeros((NDEV, 128, SK), dtype=np.int32)  # into t2_full rows
    dstf = np.full((NDEV, 128, SK), -1.0, dtype=np.float32)
    dstfR = np.full((NDEV, SK, 128), -1.0, dtype=np.float32)

    for d in range(NDEV):
        m = dev == d
        s_d, t_d, dl_d, r2_d = src[m], tloc[m], (dst[m] % NPD) % 128, t2row[m]
        for t in range(NT):
            mt = t_d == t
            s_t = s_d[mt]
            dl_t = dl_d[mt]
            r2_t = r2_d[mt]
            o = offs[t]
            j = np.arange(len(s_t))
            xidx[d, j % 128, o + j // 128] = s_t
            x2idx[d, j % 128, o + j // 128] = r2_t
            dstf[d, j % 128, o + j // 128] = dl_t.astype(np.float32)
            dstfR[d, o + j // 128, j % 128] = dl_t.astype(np.float32)

    batchf = np.full((NDEV, 128, NT), -1.0, dtype=np.float32)
    b_np = np.asarray(batch).astype(np.int64)
    for d in range(NDEV):
        for t in range(NT):
            rows = min(128, NPD - t * 128)
            g = b_np[d * NPD + t * 128: d * NPD + t * 128 + rows]
            batchf[d, :rows, t] = g.astype(np.float32)

    return Ks, offs, SK, xidx, x2idx, dstf, dstfR, batchf


def _build(Ks, offs, SK):
    """Emit the Bass program (identical for all 8 cores)."""
    nc = bacc.Bacc("TRN2", target_bir_lowering=False, debug=False, num_devices=NDEV)

    # ---- I/O ----
    xa_t = nc.dram_tensor("xa", [N, XAW], BF16, kind="ExternalInput")
    xlocT_t = nc.dram_tensor("xlocT", [C1IN, NPD], BF16, kind="ExternalInput")
    W1_t = nc.dram_tensor("W1", [C1IN, HC1], BF16, kind="ExternalInput")
    W2c_t = nc.dram_tensor("W2c", [C2IN, W2C], BF16, kind="ExternalInput")
    wa1_t = nc.dram_tensor("wa1", [C1IN, 8], BF16, kind="ExternalInput")
    b1_t = nc.dram_tensor("b1", [C1], BF16, kind="ExternalInput")
    b2_t = nc.dram_tensor("b2", [C2], F32, kind="ExternalInput")
    fcW_t = nc.dram_tensor("fcW", [C2, 2], F32, kind="ExternalInput")
    fcb_t = nc.dram_tensor("fcb", [2], F32, kind="ExternalInput")
    xidx_t = nc.dram_tensor("xidx", [128, SK], I32, kind="ExternalInput")
    x2idx_t = nc.dram_tensor("x2idx", [128, SK], I32, kind="ExternalInput")
    dstf_t = nc.dram_tensor("dstf", [128, SK], F32, kind="ExternalInput")
    dstfR_t = nc.dram_tensor("dstfR", [SK, 128], F32, kind="ExternalInput")
    batchf_t = nc.dram_tensor("batchf", [128, NT], F32, kind="ExternalInput")
    y_t = nc.dram_tensor("y", [B, 2], F32, kind="ExternalOutput")

    # ---- internal DRAM ----
    asad1_loc = nc.dram_tensor("asad1_loc", [NPD, 8], BF16)
    asad1_full = nc.dram_tensor("asad1_full", [N, 8], BF16, addr_space="Shared")
    t2_loc = nc.dram_tensor("t2_loc", [NT * 128, TW2], BF16)
    t2_full = nc.dram_tensor("t2_full", [NG * NDEV * GT * 128, TW2], BF16,
                             addr_space="Shared")
    rdscr = nc.dram_tensor("rdscr", [NT, 512], F32)
    pc_loc = nc.dram_tensor("pc_loc", [B, C2 + 1], F32)
    pc_red = nc.dram_tensor("pc_red", [B, C2 + 1], F32, addr_space="Shared")

    RG = [list(range(NDEV))]
    KMAX = max(Ks)

    import os as _os
    DEBUG = bool(int(_os.environ.get("BASS_GAT_DEBUG", "0")))
    if DEBUG:
        dbg_h1 = nc.dram_tensor("dbg_h1", [NT * 128, C1], F32, kind="ExternalOutput")
        dbg_t2 = nc.dram_tensor("dbg_t2", [NG * NDEV * GT * 128, TW2], F32,
                                kind="ExternalOutput")
        dbg_pc = nc.dram_tensor("dbg_pc", [B, C2 + 1], F32, kind="ExternalOutput")

    with tile.TileContext(nc) as tc:
        with (
            tc.tile_pool(name="const", bufs=1) as cp,
            tc.tile_pool(name="small", bufs=2) as sp,
            tc.tile_pool(name="selp", bufs=2) as selp,
            tc.tile_pool(name="selwp", bufs=3) as selwp,
            tc.tile_pool(name="xgp", bufs=2) as xgp,
            tc.tile_pool(name="utp", bufs=2) as utp,
            tc.tile_pool(name="t2p", bufs=2) as t2p,
        ):
            # ================= constants =================
            ident = cp.tile([128, 128], F32, tag="ident")
            make_identity(nc, ident[:])
            identb = cp.tile([128, 128], BF16, tag="identb")
            make_identity(nc, identb[:])
            iota_i = cp.tile([128, 128], I32, tag="iota_i")
            nc.gpsimd.iota(iota_i[:], pattern=[[1, 128]], base=0, channel_multiplier=0)
            iotaT = cp.tile([128, 128], F32, tag="iotaT")
            nc.vector.tensor_copy(out=iotaT[:], in_=iota_i[:])
            iota_ci = cp.tile([128, 1], I32, tag="iota_ci")
            nc.gpsimd.iota(iota_ci[:], pattern=[[1, 1]], base=0, channel_multiplier=1)
            iotaC = cp.tile([128, 1], F32, tag="iotaC")
            nc.vector.tensor_copy(out=iotaC[:], in_=iota_ci[:])
            ones1 = cp.tile([1, 128], BF16, tag="ones1")
            nc.vector.memset(ones1[:], 1.0)
            ones1f = cp.tile([1, 128], F32, tag="ones1f")
            nc.vector.memset(ones1f[:], 1.0)

            b1_sb = cp.tile([1, C1], BF16, tag="b1")
            nc.scalar.dma_start(out=b1_sb[:], in_=b1_t[None, :])
            b2bc = cp.tile([128, C2], F32, tag="b2bc")
            nc.scalar.dma_start(out=b2bc[:], in_=b2_t[None, :].to_broadcast([128, C2]))
            fcb_sb = cp.tile([1, 2], F32, tag="fcb")
            nc.scalar.dma_start(out=fcb_sb[:], in_=fcb_t[None, :])
            fcW_sb = cp.tile([128, 4], F32, tag="fcW")
            for c in range(2):
                nc.scalar.dma_start(out=fcW_sb[:, 2 * c:2 * c + 2],
                                  in_=fcW_t[c * 128:(c + 1) * 128, :])

            W1_sb = cp.tile([128, CC1 * HC1], BF16, tag="W1")
            for c in range(CC1):
                nc.scalar.dma_start(out=W1_sb[:, c * HC1:(c + 1) * HC1],
                                  in_=W1_t[c * 128:(c + 1) * 128, :])
            W2c_sb = cp.tile([128, CC2 * W2C], BF16, tag="W2c")
            for c in range(CC2):
                nc.scalar.dma_start(out=W2c_sb[:, c * W2C:(c + 1) * W2C],
                                  in_=W2c_t[c * 128:(c + 1) * 128, :])
            wa1_sb = cp.tile([128, CC1 * 8], BF16, tag="wa1")
            nc.scalar.dma_start(
                out=wa1_sb[:].rearrange("p (c j) -> p c j", j=8),
                in_=wa1_t[:].rearrange("(c p) j -> p c j", p=128))
            batchf_sb = cp.tile([128, NT], F32, tag="batchf")
            nc.scalar.dma_start(out=batchf_sb[:], in_=batchf_t[:, :])
            poolacc = cp.tile([128, C2 + 1], F32, tag="poolacc")
            nc.vector.memset(poolacc[:], 0.0)

            # ============ asad1 = x_loc @ wa1 ============
            with (
                tc.tile_pool(name="prep", bufs=2) as pp,
                tc.tile_pool(name="pshp", bufs=2, space="PSUM") as pshp,
            ):
                for t in range(NT):
                    rows = min(128, NPD - t * 128)
                    xT = pp.tile([128, CC1 * 128], BF16, tag="xT")
                    nc.sync.dma_start(
                        out=xT[:, :CC1 * rows].rearrange("p (c n) -> p c n", c=CC1),
                        in_=xlocT_t[:, t * 128: t * 128 + rows].rearrange(
                            "(c p) n -> p c n", p=128))
                    ps = pshp.tile([128, 8], F32, tag="ps")
                    for c in range(CC1):
                        nc.tensor.matmul(out=ps[:rows, :],
                                         lhsT=xT[:, c * rows:(c + 1) * rows],
                                         rhs=wa1_sb[:, c * 8:(c + 1) * 8],
                                         start=(c == 0), stop=(c == CC1 - 1))
                    as1 = pp.tile([128, 8], BF16, tag="as1")
                    nc.vector.tensor_copy(out=as1[:rows, :], in_=ps[:rows, :])
                    nc.sync.dma_start(out=asad1_loc[t * 128: t * 128 + rows, :],
                                      in_=as1[:rows, :])

            nc.gpsimd.collective_compute(
                "AllGather", mybir.AluOpType.bypass, replica_groups=RG,
                ins=[asad1_loc[:, :]], outs=[asad1_full[:, :]])

            # ================= layer 1 sweep =================
            with (
                tc.tile_pool(name="psu", bufs=1, space="PSUM") as psu,
                tc.tile_pool(name="psh", bufs=1, space="PSUM") as psh,
                tc.tile_pool(name="psr", bufs=1, space="PSUM") as psr,
            ):
                for t in range(NT):
                    K = Ks[t]
                    o = offs[t]
                    rows = min(128, NPD - t * 128)

                    idx_sb = sp.tile([128, KMAX], I32, tag="idx")
                    nc.gpsimd.dma_start(out=idx_sb[:, :K], in_=xidx_t[:, o:o + K])
                    dstf_sb = sp.tile([128, KMAX], F32, tag="dstf")
                    nc.gpsimd.dma_start(out=dstf_sb[:, :K], in_=dstf_t[:, o:o + K])
                    ad_sb = sp.tile([128, 4], BF16, tag="ad")
                    if rows < 128:
                        nc.vector.memset(ad_sb[:], 0.0)
                    nc.gpsimd.dma_start(out=ad_sb[:rows, :],
                                      in_=asad1_loc[t * 128:t * 128 + rows, 4:8])

                    # ---------- gather x rows + asad rows (batched) ----------
                    xg = xgp.tile([128, KMAX * XAW], BF16, tag="xg")
                    for k in range(K):
                        nc.gpsimd.indirect_dma_start(
                            out=xg[:, k * XAW:(k + 1) * XAW], out_offset=None,
                            in_=xa_t[:, :],
                            in_offset=IndirectOffsetOnAxis(
                                ap=idx_sb[:, k:k + 1], axis=0))
                    xg3 = xg[:, :K * XAW].rearrange("p (k d) -> p k d", d=XAW)
                    ag = sp.tile([128, KMAX * 8], BF16, tag="ag")
                    for k in range(K):
                        nc.gpsimd.indirect_dma_start(
                            out=ag[:, k * 8:(k + 1) * 8], out_offset=None,
                            in_=asad1_full[:, :],
                            in_offset=IndirectOffsetOnAxis(
                                ap=idx_sb[:, k:k + 1], axis=0))

                    # ---------- selection matrices (batched) ----------
                    dstfT = sp.tile([128, KMAX * 128], F32, tag="dstfT")
                    nc.scalar.dma_start(
                        out=dstfT[:, :K * 128].rearrange("p (k e) -> p k e", e=128),
                        in_=dstfR_t[None, o:o + K, :].to_broadcast([128, K, 128]))
                    selT = selp.tile([128, KMAX * 128], BF16, tag="selT")
                    nc.vector.tensor_tensor(
                        out=selT[:, :K * 128].rearrange("p (k e) -> p k e", e=128),
                        in0=iotaC[:, :, None].to_broadcast([128, K, 128]),
                        in1=dstfT[:, :K * 128].rearrange("p (k e) -> p k e", e=128),
                        op=mybir.AluOpType.is_equal)
                    sel = selp.tile([128, KMAX * 128], BF16, tag="sel")
                    nc.vector.tensor_tensor(
                        out=sel[:, :K * 128].rearrange("p (k d) -> p k d", d=128),
                        in0=dstf_sb[:, :K, None].to_broadcast([128, K, 128]),
                        in1=iotaT[:, None, :].to_broadcast([128, K, 128]),
                        op=mybir.AluOpType.is_equal)

                    # ---------- e = lrelu(as+ad); ex = exp ----------
                    ade_ps = psh.tile([128, 4 * KMAX], F32, tag="hold")
                    for k in range(K):
                        nc.tensor.matmul(out=ade_ps[:, 4 * k:4 * k + 4],
                                         lhsT=selT[:, k * 128:(k + 1) * 128],
                                         rhs=ad_sb[:],
                                         start=(k == 0), stop=(k == K - 1))
                    asg = sp.tile([128, 4 * KMAX], F32, tag="asg")
                    nc.vector.tensor_copy(out=asg[:, :4 * K].rearrange(
                        "p (k s) -> p k s", s=4),
                        in_=ag[:, :K * 8].rearrange("p (k j) -> p k j", j=8)[:, :, 0:4])
                    z = sp.tile([128, 4 * KMAX], F32, tag="z")
                    nc.vector.tensor_tensor(out=z[:, :4 * K], in0=ade_ps[:, :4 * K],
                                            in1=asg[:, :4 * K],
                                            op=mybir.AluOpType.add)
                    zs = sp.tile([128, 4 * KMAX], F32, tag="zs")
                    nc.vector.tensor_scalar_mul(zs[:, :4 * K], z[:, :4 * K], NEG)
                    nc.vector.tensor_tensor(out=z[:, :4 * K], in0=z[:, :4 * K],
                                            in1=zs[:, :4 * K],
                                            op=mybir.AluOpType.max)
                    exf = sp.tile([128, 4 * KMAX], F32, tag="exf")
                    nc.scalar.activation(out=exf[:, :4 * K], in_=z[:, :4 * K],
                                         func=mybir.ActivationFunctionType.Exp)
                    ex = sp.tile([128, 4 * KMAX], BF16, tag="ex")
                    nc.vector.tensor_copy(out=ex[:, :4 * K], in_=exf[:, :4 * K])

                    # ---------- denT[h, d] = sum_e ex[e,h] sel[e,d] ----------
                    denT_ps = psh.tile([4, 128], F32, tag="hold")
                    for k in range(K):
                        nc.tensor.matmul(out=denT_ps[:],
                                         lhsT=ex[:, 4 * k:4 * k + 4],
                                         rhs=sel[:, k * 128:(k + 1) * 128],
                                         start=(k == 0), stop=(k == K - 1))

                    # ---------- aggregate ut[f,(h,d)] += x ex ----------
                    ut_ps = psu.tile([128, CC1 * 512], F32, tag="ut")
                    for k in range(K):
                        selw = selwp.tile([128, 512], BF16, tag="selw")
                        for h in range(H):
                            nc.vector.tensor_scalar_mul(
                                selw[:, h * 128:(h + 1) * 128],
                                sel[:, k * 128:(k + 1) * 128],
                                exf[:, 4 * k + h:4 * k + h + 1])
                        for c in range(CC1):
                            nc.tensor.matmul(
                                out=ut_ps[:, c * 512:(c + 1) * 512],
                                lhsT=xg[:, k * XAW + c * 128: k * XAW + (c + 1) * 128],
                                rhs=selw[:],
                                start=(k == 0), stop=(k == K - 1))

                    # ---------- rdenT broadcast + divide ----------
                    rdT = sp.tile([4, 128], F32, tag="rdT")
                    nc.vector.tensor_scalar(out=rdT[:], in0=denT_ps[:],
                                            scalar1=4.0, scalar2=1e-16,
                                            op0=mybir.AluOpType.mult,
                                            op1=mybir.AluOpType.max)
                    nc.vector.reciprocal(out=rdT[:], in_=rdT[:])
                    nc.sync.dma_start(
                        out=rdscr[t:t + 1, :].rearrange("a (p f) -> (a p) f", p=4),
                        in_=rdT[:])
                    rdb = sp.tile([128, 512], F32, tag="rdb")
                    nc.sync.dma_start(out=rdb[:],
                                      in_=rdscr[t:t + 1, :].to_broadcast([128, 512]))
                    ut_sb = utp.tile([128, CC1 * 512], BF16, tag="ut")
                    for c in range(CC1):
                        nc.vector.tensor_tensor(out=ut_sb[:, c * 512:(c + 1) * 512],
                                                in0=ut_ps[:, c * 512:(c + 1) * 512],
                                                in1=rdb[:],
                                                op=mybir.AluOpType.mult)

                    # ---------- project + bias ----------
                    out_ps = psh.tile([128, C1], F32, tag="hold")
                    first = True
                    for h in range(H):
                        for c in range(CC1):
                            nc.tensor.matmul(
                                out=out_ps[:],
                                lhsT=ut_sb[:, c * 512 + h * 128: c * 512 + (h + 1) * 128],
                                rhs=W1_sb[:, c * HC1 + h * C1: c * HC1 + (h + 1) * C1],
                                start=first, stop=False)
                            first = False
                    nc.tensor.matmul(out=out_ps[:], lhsT=ones1[:], rhs=b1_sb[:],
                                     start=False, stop=True)
                    h1_sb = t2p.tile([128, C1], BF16, tag="h1")
                    nc.vector.tensor_copy(out=h1_sb[:], in_=out_ps[:])
                    if DEBUG:
                        h1f = sp.tile([128, C1], F32, tag="h1f")
                        nc.vector.tensor_copy(out=h1f[:], in_=out_ps[:])
                        nc.sync.dma_start(out=dbg_h1[t * 128:(t + 1) * 128, :],
                                          in_=h1f[:])

                    # ---------- xw2 = h1 @ [W2 | wa2] ----------
                    h1T = t2p.tile([128, C1], BF16, tag="h1T")
                    for c in range(CC2):
                        tp = psr.tile([128, 128], BF16, tag="rot")
                        nc.tensor.transpose(out=tp[:],
                                            in_=h1_sb[:, c * 128:(c + 1) * 128],
                                            identity=identb[:])
                        nc.vector.tensor_copy(out=h1T[:, c * 128:(c + 1) * 128],
                                              in_=tp[:])
                    xw2_sb = t2p.tile([128, TW2], BF16, tag="xw2")
                    for (lo, w) in ((0, 512), (512, 512), (1024, 8)):
                        xp = psr.tile([128, 512], F32, tag="rot")
                        for c in range(CC2):
                            nc.tensor.matmul(
                                out=xp[:, :w],
                                lhsT=h1T[:, c * 128:(c + 1) * 128],
                                rhs=W2c_sb[:, c * W2C + lo: c * W2C + lo + w],
                                start=(c == 0), stop=(c == CC2 - 1))
                        nc.vector.tensor_copy(out=xw2_sb[:, lo:lo + w], in_=xp[:, :w])
                    nc.sync.dma_start(out=t2_loc[t * 128:(t + 1) * 128, :],
                                      in_=xw2_sb[:])

                    if t % GT == GT - 1:
                        g = t // GT
                        nc.gpsimd.collective_compute(
                            "AllGather", mybir.AluOpType.bypass, replica_groups=RG,
                            ins=[t2_loc[g * GT * 128:(g + 1) * GT * 128, :]],
                            outs=[t2_full[g * NDEV * GT * 128:
                                          (g + 1) * NDEV * GT * 128, :]])

            if DEBUG:
                t2f = sp.tile([128, TW2], F32, tag="t2f")
                for i in range(NG * NDEV * GT):
                    t2b = sp.tile([128, TW2], BF16, tag="t2b")
                    nc.sync.dma_start(out=t2b[:],
                                      in_=t2_full[i * 128:(i + 1) * 128, :])
                    nc.vector.tensor_copy(out=t2f[:], in_=t2b[:])
                    nc.sync.dma_start(out=dbg_t2[i * 128:(i + 1) * 128, :],
                                      in_=t2f[:])

            # ================= layer 2 sweep + pooling =================
            with (
                tc.tile_pool(name="psu2", bufs=2, space="PSUM") as psu2,
                tc.tile_pool(name="psh2", bufs=2, space="PSUM") as psh2,
            ):
                for t in range(NT):
                    K = Ks[t]
                    o = offs[t]
                    rows = min(128, NPD - t * 128)

                    idx_sb = sp.tile([128, KMAX], I32, tag="idx")
                    nc.gpsimd.dma_start(out=idx_sb[:, :K], in_=x2idx_t[:, o:o + K])
                    dstf_sb = sp.tile([128, KMAX], F32, tag="dstf")
                    nc.gpsimd.dma_start(out=dstf_sb[:, :K], in_=dstf_t[:, o:o + K])
                    ad_sb = sp.tile([128, 4], BF16, tag="ad")
                    if rows < 128:
                        nc.vector.memset(ad_sb[:], 0.0)
                    nc.gpsimd.dma_start(out=ad_sb[:rows, :],
                                      in_=t2_loc[t * 128:t * 128 + rows,
                                                 HC2 + 4:HC2 + 8])

                    hg = xgp.tile([128, KMAX * TW2], BF16, tag="hg")
                    for k in range(K):
                        nc.gpsimd.indirect_dma_start(
                            out=hg[:, k * TW2:(k + 1) * TW2], out_offset=None,
                            in_=t2_full[:, :],
                            in_offset=IndirectOffsetOnAxis(
                                ap=idx_sb[:, k:k + 1], axis=0))
                    hg3 = hg[:, :K * TW2].rearrange("p (k d) -> p k d", d=TW2)

                    dstfT = sp.tile([128, KMAX * 128], F32, tag="dstfT")
                    nc.scalar.dma_start(
                        out=dstfT[:, :K * 128].rearrange("p (k e) -> p k e", e=128),
                        in_=dstfR_t[None, o:o + K, :].to_broadcast([128, K, 128]))
                    selT = selp.tile([128, KMAX * 128], BF16, tag="selT")
                    nc.vector.tensor_tensor(
                        out=selT[:, :K * 128].rearrange("p (k e) -> p k e", e=128),
                        in0=iotaC[:, :, None].to_broadcast([128, K, 128]),
                        in1=dstfT[:, :K * 128].rearrange("p (k e) -> p k e", e=128),
                        op=mybir.AluOpType.is_equal)
                    sel = selp.tile([128, KMAX * 128], BF16, tag="sel")
                    nc.vector.tensor_tensor(
                        out=sel[:, :K * 128].rearrange("p (k d) -> p k d", d=128),
                        in0=dstf_sb[:, :K, None].to_broadcast([128, K, 128]),
                        in1=iotaT[:, None, :].to_broadcast([128, K, 128]),
                        op=mybir.AluOpType.is_equal)

                    ade_ps = psh2.tile([128, 4 * KMAX], F32, tag="hold")
                    for k in range(K):
                        nc.tensor.matmul(out=ade_ps[:, 4 * k:4 * k + 4],
                                         lhsT=selT[:, k * 128:(k + 1) * 128],
                                         rhs=ad_sb[:],
                                         start=(k == 0), stop=(k == K - 1))
                    asg = sp.tile([128, 4 * KMAX], F32, tag="asg")
                    nc.vector.tensor_copy(out=asg[:, :4 * K].rearrange(
                        "p (k s) -> p k s", s=4),
                        in_=hg3[:, :, HC2:HC2 + 4])
                    z = sp.tile([128, 4 * KMAX], F32, tag="z")
                    nc.vector.tensor_tensor(out=z[:, :4 * K], in0=ade_ps[:, :4 * K],
                                            in1=asg[:, :4 * K],
                                            op=mybir.AluOpType.add)
                    zs = sp.tile([128, 4 * KMAX], F32, tag="zs")
                    nc.vector.tensor_scalar_mul(zs[:, :4 * K], z[:, :4 * K], NEG)
                    nc.vector.tensor_tensor(out=z[:, :4 * K], in0=z[:, :4 * K],
                                            in1=zs[:, :4 * K],
                                            op=mybir.AluOpType.max)
                    exf = sp.tile([128, 4 * KMAX], F32, tag="exf")
                    nc.scalar.activation(out=exf[:, :4 * K], in_=z[:, :4 * K],
                                         func=mybir.ActivationFunctionType.Exp)
                    ex = sp.tile([128, 4 * KMAX], BF16, tag="ex")
                    nc.vector.tensor_copy(out=ex[:, :4 * K], in_=exf[:, :4 * K])

                    # agg[d, (h,c)] += ex * xw2 ; den[d, h] += ex
                    agg_ps = psu2.tile([128, HC2], F32, tag="ut2")
                    den_ps = psh2.tile([128, 4], F32, tag="hold")
                    for k in range(K):
                        exw = selwp.tile([128, HC2], BF16, tag="exw")
                        for h in range(H):
                            nc.vector.tensor_scalar_mul(
                                exw[:, h * C2:(h + 1) * C2],
                                hg[:, k * TW2 + h * C2: k * TW2 + (h + 1) * C2],
                                exf[:, 4 * k + h:4 * k + h + 1])
                        nc.tensor.matmul(out=agg_ps[:, 0:512],
                                         lhsT=sel[:, k * 128:(k + 1) * 128],
                                         rhs=exw[:, 0:512],
                                         start=(k == 0), stop=(k == K - 1))
                        nc.tensor.matmul(out=agg_ps[:, 512:1024],
                                         lhsT=sel[:, k * 128:(k + 1) * 128],
                                         rhs=exw[:, 512:1024],
                                         start=(k == 0), stop=(k == K - 1))
                        nc.tensor.matmul(out=den_ps[:],
                                         lhsT=sel[:, k * 128:(k + 1) * 128],
                                         rhs=ex[:, 4 * k:4 * k + 4],
                                         start=(k == 0), stop=(k == K - 1))

                    rd2 = sp.tile([128, 4], F32, tag="rd2")
                    nc.vector.tensor_scalar(out=rd2[:], in0=den_ps[:],
                                            scalar1=4.0, scalar2=1e-16,
                                            op0=mybir.AluOpType.mult,
                                            op1=mybir.AluOpType.max)
                    nc.vector.reciprocal(out=rd2[:], in_=rd2[:])
                    h2acc = sp.tile([128, C2], F32, tag="h2acc")
                    nc.vector.tensor_scalar_mul(h2acc[:], agg_ps[:, 0:C2],
                                                rd2[:, 0:1])
                    for h in range(1, H):
                        nc.vector.scalar_tensor_tensor(
                            out=h2acc[:], in0=agg_ps[:, h * C2:(h + 1) * C2],
                            scalar=rd2[:, h:h + 1], in1=h2acc[:],
                            op0=mybir.AluOpType.mult, op1=mybir.AluOpType.add)
                    h2p = t2p.tile([128, C2 + 1], BF16, tag="h2p")
                    nc.vector.tensor_tensor(out=h2p[:, :C2], in0=h2acc[:],
                                            in1=b2bc[:], op=mybir.AluOpType.add)
                    nc.vector.memset(h2p[:, C2:C2 + 1], 1.0)

                    selB = sp.tile([128, 128], BF16, tag="selB")
                    nc.vector.tensor_tensor(
                        out=selB[:],
                        in0=batchf_sb[:, t:t + 1].to_broadcast([128, 128]),
                        in1=iotaT[:], op=mybir.AluOpType.is_equal)
                    pc_ps = psh2.tile([128, C2 + 1], F32, tag="hold")
                    nc.tensor.matmul(out=pc_ps[:], lhsT=selB[:], rhs=h2p[:],
                                     start=True, stop=True)
                    nc.vector.tensor_tensor(out=poolacc[:], in0=poolacc[:],
                                            in1=pc_ps[:], op=mybir.AluOpType.add)

                # ================= pool reduce + FC =================
                nc.sync.dma_start(out=pc_loc[:, :], in_=poolacc[:])
                nc.gpsimd.collective_compute(
                    "AllReduce", mybir.AluOpType.add, replica_groups=RG,
                    ins=[pc_loc[:, :]], outs=[pc_red[:, :]])
                pc_sb = sp.tile([128, C2 + 1], F32, tag="pc")
                nc.sync.dma_start(out=pc_sb[:], in_=pc_red[:, :])
                if DEBUG:
                    nc.sync.dma_start(out=dbg_pc[:, :], in_=pc_red[:, :])
                cnt = sp.tile([128, 1], F32, tag="cnt")
                nc.vector.tensor_scalar_max(cnt[:], pc_sb[:, C2:C2 + 1], 1.0)
                nc.vector.reciprocal(out=cnt[:], in_=cnt[:])
                g_sb = sp.tile([128, C2], F32, tag="g")
                nc.vector.tensor_scalar_mul(g_sb[:], pc_sb[:, :C2], cnt[:, :1])

                y_ps = psh2.tile([128, 2], F32, tag="hold")
                for c in range(2):
                    tp = psu2.tile([128, 128], F32, tag="ut2")
                    nc.tensor.transpose(out=tp[:], in_=g_sb[:, c * 128:(c + 1) * 128],
                                        identity=ident[:])
                    gT = sp.tile([128, 128], F32, tag="gT")
                    nc.vector.tensor_copy(out=gT[:], in_=tp[:])
                    nc.tensor.matmul(out=y_ps[:], lhsT=gT[:],
                                     rhs=fcW_sb[:, 2 * c:2 * c + 2],
                                     start=(c == 0), stop=False)
                nc.tensor.matmul(out=y_ps[:], lhsT=ones1f[:], rhs=fcb_sb[:],
                                 start=False, stop=True)
                y_sb = sp.tile([128, 2], F32, tag="y")
                nc.vector.tensor_copy(out=y_sb[:], in_=y_ps[:])
                nc.sync.dma_start(out=y_t[:, :], in_=y_sb[:])

    nc.compile()
    return nc


_CACHE = {}


def kernel(**inputs):
    x = np.ascontiguousarray(np.asarray(inputs["x"], dtype=np.float32))
    edge_index = np.asarray(inputs["edge_index"])
    batch = np.asarray(inputs["batch"])
    W1 = np.asarray(inputs["W1"], dtype=np.float32)
    W2 = np.asarray(inputs["W2"], dtype=np.float32)
    a_src1 = np.asarray(inputs["a_src1"], dtype=np.float32)
    a_dst1 = np.asarray(inputs["a_dst1"], dtype=np.float32)
    a_src2 = np.asarray(inputs["a_src2"], dtype=np.float32)
    a_dst2 = np.asarray(inputs["a_dst2"], dtype=np.float32)
    b1 = np.asarray(inputs["b1"], dtype=np.float32)
    b2 = np.asarray(inputs["b2"], dtype=np.float32)
    fcW = np.ascontiguousarray(np.asarray(inputs["fcW"], dtype=np.float32))
    fcb = np.asarray(inputs["fcb"], dtype=np.float32)

    Ks, offs, SK, xidx, x2idx, dstf, dstfR, batchf = _host_prep(edge_index, batch)

    key = (tuple(Ks),)
    if key not in _CACHE:
        _CACHE[key] = _build(Ks, offs, SK)
    nc = _CACHE[key]

    # weight-only prep: wa = W^T a per head (folded attention projections)
    wa1 = np.zeros((C1IN, 8), dtype=np.float32)
    wa2 = np.zeros((C2IN, 8), dtype=np.float32)
    for h in range(H):
        wa1[:, h] = W1[:, h * C1:(h + 1) * C1] @ a_src1[h]
        wa1[:, 4 + h] = W1[:, h * C1:(h + 1) * C1] @ a_dst1[h]
        wa2[:, h] = W2[:, h * C2:(h + 1) * C2] @ a_src2[h]
        wa2[:, 4 + h] = W2[:, h * C2:(h + 1) * C2] @ a_dst2[h]
    W2c = np.concatenate([W2, wa2], axis=1)

    xa = np.zeros((N, XAW), dtype=BFNP)
    xa[:, :C1IN] = x.astype(BFNP)

    in_maps = []
    for d in range(NDEV):
        xloc = x[d * NPD:(d + 1) * NPD]
        in_maps.append({
            "xa": xa,
            "xlocT": np.ascontiguousarray(xloc.T).astype(BFNP),
            "W1": W1.astype(BFNP), "W2c": W2c.astype(BFNP),
            "wa1": wa1.astype(BFNP), "b1": b1.astype(BFNP), "b2": b2,
            "fcW": fcW, "fcb": fcb,
            "xidx": xidx[d], "x2idx": x2idx[d], "dstf": dstf[d],
            "dstfR": dstfR[d], "batchf": batchf[d],
        })

    import os as _os
    trace = bool(int(_os.environ.get("BASS_GAT_TRACE", "0")))
    kwargs = {}
    if trace:
        _setup_ntff_hook()
        kwargs = dict(trace=True, trace_cores=[0])
    res = run_bass_kernel_spmd(nc, in_maps, core_ids=list(range(NDEV)), **kwargs)
    if trace:
        kernel.last_exec_ns = res.exec_time_ns
        kernel.last_trace = res.instructions_and_trace
        if res.exec_time_ns is not None:
            print(f"HW exec time: {res.exec_time_ns} ns")
    if bool(int(_os.environ.get("BASS_GAT_DEBUG", "0"))):
        kernel.debug_results = res.results
    return res.results[0]["y"]


def _setup_ntff_hook():
    """The image's antenv lacks axon_hooks; synthesize it and register the
    ctypes NTFF profiling hook so trace=True works."""
    import types
    import antenv
    if hasattr(antenv, "axon_hooks"):
        return
    mod = types.ModuleType("antenv.axon_hooks")
    state = {"hook": None}
    mod.set_axon_ntff_profile_hook = lambda h: state.__setitem__("hook", h)
    mod.get_axon_ntff_profile_hook = lambda: state["hook"]
    sys.modules["antenv.axon_hooks"] = mod
    antenv.axon_hooks = mod
    try:
        from trn_agent_boot.trn_boot import _ntff_profile_via_ctypes
        hook = _ntff_profile_via_ctypes("/opt/axon/libaxon_pjrt.so")
        mod.set_axon_ntff_profile_hook(hook)
    except Exception as e:
        print("ntff hook setup failed:", e)
